# revision 1
# baseline (speedup 1.0000x reference)
"""CrossAttention kernel for 8 Trainium2 NeuronCores.

Reference computation (per batch element b):
    q = ts[b] @ q_w.T + q_b          # [512, 1024]
    k = llm[b] @ k_w.T + k_b         # [2048, 1024]
    v = llm[b] @ v_w.T + v_b         # [2048, 1024]
    per head h (16 heads x 64 dims):
        scores = q_h @ k_h.T / 8     # [512, 2048]
        attn = softmax(scores, -1)
        ctx_h = attn @ v_h           # [512, 64]
    out = ctx @ o_w.T + o_b          # [512, 1024]

Sharding: data-parallel over batch (B=8 -> one element per core), no
collectives.

Per-core structure (PE matmul cost on TRN2 is proportional ONLY to the
output free-dim size, so every matmul is oriented to keep the moving
dim minimal):

  QT[j, p]   = q_w @ ts.T  + q_b      (feature-major, bias per-partition)
  KT[j, s]   = k_w @ llm.T + k_b
  V'[s, j']  = llm @ v_w.T + v_b      (j' = 16 heads x 65 cols; col 64 of
                                       each head block is ones -> denom)
  scoresT_h[s, p] = KT_h.T @ QT_h     (fp8e4m3 DoubleRow matmul, 0.5
                                       cycles/row; qt/kt cast to fp8 by the
                                       projection bias-add, then repacked to
                                       the [32, 2, *] DoubleRow layout by
                                       SBUF->SBUF DMAs on the idle
                                       Pool/SWDGE queue; adds ~1e-2 rel err,
                                       gate is 2e-2)
  expT = exp(scoresT / 8)             (no max subtraction: |scores/8| < ~3)
  ctx_h[p-chunk, 0:65] = expT_h(:,pc).T @ V'_h   accumulated over s-tiles
                                      (out free = 65, NOT 512; col 64 =
                                       softmax denominator)
  ctx_nat[p, d] = ctx * (1/denom)     (per-partition scalar mul on DVE)
  cxT[d, p]  = PE-transpose(ctx_nat)  (identity matmul, 128x128 blocks)
  out[p, j]  = cxT.T @ o_wT + o_b     (d<7 partials pre-accumulated during
                                       the last head pair with the bias
                                       folded in; the tail is just the d=7
                                       matmul, an identity-matmul that
                                       accumulates the bf16 partial into
                                       PSUM, an Act-engine copy, and DMA)

Emission is software-pipelined: ctx matmuls lag scores/exp by one
stage so PE never waits in-order on the Act engine; K-projection
groups for head-pair p+1 are spread through pair p's stages as PE
filler; O-projection partials fill pair 7.

All matmuls bf16 inputs / fp32 PSUM accumulate.  Host does layout-only
prep (transpose, bf16 cast, bias broadcast).
"""
import numpy as np
import ml_dtypes

D = 1024          # d_model
P = 512           # ts sequence length
S = 2048          # llm sequence length
H = 16            # heads
DH = 64           # head dim
NCORES = 8
NDT = D // 128    # 8 d-tiles
NST = S // 128    # 16 s-tiles
NPT = P // 128    # 4 p-tiles
NPAIR = H // 2    # 8 head pairs

_BF16 = ml_dtypes.bfloat16

_cached_nc = None


def _build_nc():
    import concourse.tile as tile
    from concourse import bacc, mybir

    f32 = mybir.dt.float32
    bf16 = mybir.dt.bfloat16

    nc = bacc.Bacc("TRN2", target_bir_lowering=False, debug=False,
                   num_devices=NCORES)

    tsT = nc.declare_dram_parameter("tsT", [D, P], bf16, isOutput=False)
    llmT = nc.declare_dram_parameter("llmT", [D, S], bf16, isOutput=False)
    qwT = nc.declare_dram_parameter("qwT", [D, D], bf16, isOutput=False)
    kwT = nc.declare_dram_parameter("kwT", [D, D], bf16, isOutput=False)
    vwT = nc.declare_dram_parameter("vwT", [D, D], bf16, isOutput=False)
    owT = nc.declare_dram_parameter("owT", [D, D], bf16, isOutput=False)
    qkb = nc.declare_dram_parameter("qkb", [128, 2 * NDT], f32, isOutput=False)
    vbb = nc.declare_dram_parameter("vbb", [128, D], bf16, isOutput=False)
    obb = nc.declare_dram_parameter("obb", [128, D], bf16, isOutput=False)
    out = nc.declare_dram_parameter("out", [P, D], bf16, isOutput=True)

    with tile.TileContext(nc) as tc:
        _emit(tc, nc, tile, mybir, f32, bf16,
              tsT, llmT, qwT, kwT, vwT, owT, qkb, vbb, obb, out)
    nc.compile()
    return nc


def _emit(tc, nc, tile, mybir, f32, bf16,
          tsT, llmT, qwT, kwT, vwT, owT, qkb, vbb, obb, out):
    from contextlib import ExitStack
    from concourse.masks import make_identity

    Exp = mybir.ActivationFunctionType.Exp
    f8 = mybir.dt.float8e4
    DR = mybir.MatmulPerfMode.DoubleRow

    with ExitStack() as ctx:
        persist = ctx.enter_context(tc.tile_pool(name="persist", bufs=1))
        wpool = ctx.enter_context(tc.tile_pool(name="wpool", bufs=32))
        ktpool = ctx.enter_context(tc.tile_pool(name="ktpool", bufs=2))
        ktdrpool = ctx.enter_context(tc.tile_pool(name="ktdrpool", bufs=3))
        qtdrpool = ctx.enter_context(tc.tile_pool(name="qtdrpool", bufs=3))
        expool = ctx.enter_context(tc.tile_pool(name="expool", bufs=6))
        rpool = ctx.enter_context(tc.tile_pool(name="rpool", bufs=2))
        opool = ctx.enter_context(tc.tile_pool(name="opool", bufs=5))

        # identity for PE transposes (gpsimd, off the critical engines)
        ident = persist.tile([128, 128], bf16, name="ident", tag="ident")
        make_identity(nc, ident)

        # ---- input DMAs, per-d tiles in consumption order so PE can
        # stream each d-accumulation group behind the DMA arrivals.
        # ts/qw pairwise-interleaved: QT jt0's d-th matmul needs only
        # (ts_d, qw_d), so the first matmul fires ~4us in instead of
        # waiting for all 3MB.  Then kw, llm h0, vw, llm h1, biases, ow.
        qkb_sb = persist.tile([128, 2 * NDT], f32, name="qkb_sb", tag="qkb_sb")
        ts_sb = []
        qw_sb = []
        for d in range(NDT):
            t = persist.tile([128, P], bf16, name=f"ts_sb{d}", tag=f"ts_sb{d}")
            nc.sync.dma_start(out=t, in_=tsT.ap()[d * 128:(d + 1) * 128, :])
            ts_sb.append(t)
            w = wpool.tile([128, D], bf16, name=f"qw_sb{d}", tag="w")
            nc.sync.dma_start(out=w, in_=qwT.ap()[d * 128:(d + 1) * 128, :])
            qw_sb.append(w)
            if d == 0:
                # qkb after the first ts/qw pair: keeps HWDGE slot #1 for
                # the first matmul's inputs, still far ahead of the bias adds
                nc.sync.dma_start(out=qkb_sb, in_=qkb.ap())

        def load_w(dram, prefix):
            tiles = []
            for d in range(NDT):
                t = wpool.tile([128, D], bf16, name=f"{prefix}{d}", tag="w")
                nc.sync.dma_start(out=t, in_=dram.ap()[d * 128:(d + 1) * 128, :])
                tiles.append(t)
            return tiles

        kw_sb = load_w(kwT, "kw_sb")
        # llm as 16 half-tiles [128, 1024]: llm_sb[d][h] covers s-cols
        # h*1024 .. h*1024+1024.  h0 first (KT sc0/sc1 + V' st<8), then vw,
        # then h1 -- matches PE consumption order.
        llm_sb = [[None, None] for _ in range(NDT)]
        for d in range(NDT):
            t = persist.tile([128, 1024], bf16, name=f"llm_sb{d}h0",
                             tag=f"llm_sb{d}h0")
            nc.sync.dma_start(out=t, in_=llmT.ap()[d * 128:(d + 1) * 128,
                                                   0:1024])
            llm_sb[d][0] = t
        vbb_sb = persist.tile([128, D], bf16, name="vbb_sb", tag="vbb_sb")
        nc.sync.dma_start(out=vbb_sb, in_=vbb.ap())
        vw_sb = load_w(vwT, "vw_sb")
        for d in range(NDT):
            t = persist.tile([128, 1024], bf16, name=f"llm_sb{d}h1",
                             tag=f"llm_sb{d}h1")
            nc.sync.dma_start(out=t, in_=llmT.ap()[d * 128:(d + 1) * 128,
                                                   1024:2048])
            llm_sb[d][1] = t
        obb_sb = persist.tile([128, D], bf16, name="obb_sb", tag="obb_sb")
        nc.sync.dma_start(out=obb_sb, in_=obb.ap())
        ow_sb = load_w(owT, "ow_sb")

        # ---- persistent intermediates ----
        qt_sb = []
        for jt in range(NDT):
            qt_sb.append(persist.tile([128, P], f8, name=f"qt_sb{jt}",
                                      tag=f"qt_sb{jt}"))
        qt_dr = [None] * NDT
        kt_sb = [None] * NDT
        kt_dr = [None] * NDT
        vp_sb = [None] * NST
        # ctx_nat: [p(128), pt(4) x d(1024)] bf16 -- normalized context in
        # natural layout, all four p-tiles side by side.
        ctx_nat = persist.tile([128, NPT * D], bf16, name="ctx_nat",
                               tag="ctx_nat")
        cxT = []
        for d in range(NDT):
            cxT.append(persist.tile([128, P], bf16, name=f"cxT{d}",
                                    tag=f"cxT{d}"))
        partial = []
        for T in range(8):
            partial.append(persist.tile([128, 512], bf16, name=f"opart{T}",
                                        tag=f"opart{T}"))

        with tc.tile_pool(name="psS", bufs=2, space="PSUM") as psS, \
             tc.tile_pool(name="psC", bufs=2, space="PSUM") as psC, \
             tc.tile_pool(name="psP", bufs=2, space="PSUM") as psP:

            # ---------------- emission helpers ----------------
            def emit_qt(jt):
                ps = psP.tile([128, P], f32, name=f"ps_q{jt}", tag="psP")
                for d in range(NDT):
                    nc.tensor.matmul(
                        ps,
                        lhsT=qw_sb[d][:, jt * 128:(jt + 1) * 128],
                        rhs=ts_sb[d],
                        start=(d == 0), stop=(d == NDT - 1))
                nc.vector.tensor_scalar_add(qt_sb[jt], ps,
                                            qkb_sb[:, jt:jt + 1])

            def emit_dr_repack(jt, lo, hi):
                # kt_dr[u*32+p, i, s] = kt[u*64+i*32+p, s] (fp8, partition
                # remap via SBUF->SBUF DMA on the idle Pool/SWDGE queue);
                # same for qt_dr when lo==0 and hi==S is not required.
                k3 = kt_dr[jt].rearrange("q (i s) -> q i s", i=2)
                for u in range(2):
                    for i in range(2):
                        nc.gpsimd.dma_start(
                            out=k3[u * 32:(u + 1) * 32, i:i + 1, lo:hi],
                            in_=kt_sb[jt][u * 64 + i * 32:u * 64 + i * 32 + 32,
                                          lo:hi])

            def emit_qt_repack(jt):
                if qt_dr[jt] is None:
                    qt_dr[jt] = qtdrpool.tile([64, 2 * P], f8,
                                              name=f"qt_dr{jt}", tag="qtdr")
                q3 = qt_dr[jt].rearrange("q (i x) -> q i x", i=2)
                for u in range(2):
                    for i in range(2):
                        nc.gpsimd.dma_start(
                            out=q3[u * 32:(u + 1) * 32, i:i + 1, :],
                            in_=qt_sb[jt][u * 64 + i * 32:
                                          u * 64 + i * 32 + 32, :])

            def emit_kt_sc(jt, sc):
                # KT[j, s] for one 512-col s-chunk; llm half h = sc//2.
                if kt_sb[jt] is None:
                    kt_sb[jt] = ktpool.tile([128, S], f8,
                                            name=f"kt_sb{jt}", tag="kt")
                    kt_dr[jt] = ktdrpool.tile([64, 2 * S], f8,
                                              name=f"kt_dr{jt}", tag="ktdr")
                h, c = sc // 2, sc % 2
                ps = psP.tile([128, 512], f32, name=f"ps_k{jt}_{sc}",
                              tag="psP")
                for d in range(NDT):
                    nc.tensor.matmul(
                        ps,
                        lhsT=kw_sb[d][:, jt * 128:(jt + 1) * 128],
                        rhs=llm_sb[d][h][:, c * 512:(c + 1) * 512],
                        start=(d == 0), stop=(d == NDT - 1))
                nc.vector.tensor_scalar_add(
                    kt_sb[jt][:, sc * 512:(sc + 1) * 512], ps,
                    qkb_sb[:, NDT + jt:NDT + jt + 1])
                if jt == 0 and sc >= 2:
                    # sc0/sc1 are consumed by pair-0 stages k<4 via the
                    # non-DR path below, so their repacks are skipped.
                    emit_dr_repack(0, sc * 512, (sc + 1) * 512)
                elif jt >= 1 and sc == 3:
                    emit_dr_repack(jt, 0, S)

            def emit_v(st):
                # V'[s, h*65 + x]: x<64 -> v_h columns, x=64 -> ones
                vp = persist.tile([128, H * (DH + 1)], bf16,
                                  name=f"vp_sb{st}", tag=f"vp_sb{st}")
                vp3 = vp.rearrange("p (h x) -> p h x", x=DH + 1)
                nc.vector.memset(vp3[:, :, DH:DH + 1], 1.0)
                h, c = st // 8, st % 8
                for jc in range(2):
                    ps = psP.tile([128, 512], f32, name=f"ps_v{st}_{jc}",
                                  tag="psP")
                    for d in range(NDT):
                        nc.tensor.matmul(
                            ps,
                            lhsT=llm_sb[d][h][:, c * 128:(c + 1) * 128],
                            rhs=vw_sb[d][:, jc * 512:(jc + 1) * 512],
                            start=(d == 0), stop=(d == NDT - 1))
                    nc.vector.tensor_add(
                        vp3[:, jc * 8:(jc + 1) * 8, 0:DH],
                        ps.rearrange("p (h x) -> p h x", x=DH),
                        vbb_sb[:, jc * 512:(jc + 1) * 512]
                        .rearrange("p (h x) -> p h x", x=DH))
                vp_sb[st] = vp

            emitted_v = [0]

            def ensure_v(upto):
                while emitted_v[0] <= upto:
                    emit_v(emitted_v[0])
                    emitted_v[0] += 1

            def emit_ctx(p, k, ets):
                # ctx[p-chunk, 0:65] += expT_h(st).T @ V'_h, out free = 65.
                # PSUM zero-region semantics: start=True on the FIRST matmul
                # of each psc bank marks the whole 2KB region pending-zero;
                # later chunks' first writes replace-if-pending, so only
                # (st==0, pc==0) starts and only (st==15, pc==3) stops.
                for i in range(2):
                    st = 2 * k + i
                    for u in range(2):
                        h = 2 * p + u
                        for pc in range(NPT):
                            nc.tensor.matmul(
                                psc[u][:, pc * (DH + 1):
                                       (pc + 1) * (DH + 1)],
                                lhsT=ets[u][:, i * 512 + pc * 128:
                                            i * 512 + (pc + 1) * 128],
                                rhs=vp_sb[st][:, h * (DH + 1):
                                              (h + 1) * (DH + 1)],
                                start=(st == 0 and pc == 0),
                                stop=(st == NST - 1 and pc == NPT - 1))

            def emit_normalize(p):
                # On the last pair the Act engine is done with exps, so half
                # the muls go there to shorten the tail's serial chain.
                split = (p == NPAIR - 1)
                rcs = []
                for u in range(2):
                    h = 2 * p + u
                    psc3 = psc[u].rearrange("p (c x) -> p c x", x=DH + 1)
                    rc = rpool.tile([128, NPT], f32, name=f"rc{h}", tag="rc")
                    rc3 = rc.rearrange("p (c x) -> p c x", x=1)
                    nc.vector.reciprocal(rc3, psc3[:, :, DH:DH + 1])
                    rcs.append(rc)
                for pc in range(NPT):
                    for u in range(2):
                        h = 2 * p + u
                        dst = ctx_nat[:, pc * D + h * DH:pc * D + (h + 1) * DH]
                        srcp = psc[u][:, pc * (DH + 1):pc * (DH + 1) + DH]
                        if split and u == 1:
                            nc.scalar.mul(dst, srcp, rcs[u][:, pc:pc + 1])
                        else:
                            nc.vector.tensor_scalar_mul(
                                dst, srcp, rcs[u][:, pc:pc + 1])

            def emit_transposes(p):
                # cxT[p][d, p_global] from ctx_nat pair-p column blocks.
                for pc in range(NPT):
                    pst = psP.tile([128, 128], bf16, name=f"pst{p}_{pc}",
                                   tag="psP")
                    nc.tensor.transpose(
                        pst,
                        ctx_nat[:, pc * D + p * 128:pc * D + (p + 1) * 128],
                        ident)
                    nc.vector.tensor_copy(cxT[p][:, pc * 128:(pc + 1) * 128],
                                          pst)

            def emit_opartial(T):
                # out tile T=(pt,jc): sum d=0..6 plus output bias -> bf16.
                pt, jc = T // 2, T % 2
                ps = psP.tile([128, 512], f32, name=f"ps_op{T}", tag="psP")
                for d in range(NDT - 2):
                    nc.tensor.matmul(
                        ps, lhsT=cxT[d][:, pt * 128:(pt + 1) * 128],
                        rhs=ow_sb[d][:, jc * 512:(jc + 1) * 512],
                        start=(d == 0), stop=(d == NDT - 3))
                nc.vector.tensor_add(partial[T], ps,
                                     obb_sb[:, jc * 512:(jc + 1) * 512])

            # ---------------- phase A ----------------
            for jt in range(NDT):
                emit_qt(jt)
            emit_qt_repack(0)
            emit_kt_sc(0, 0)
            emit_kt_sc(0, 1)

            # ---------------- pipelined head pairs ----------------
            # Stage (p, k) covers s-tiles 2k, 2k+1 of pair p.  ctx lags one
            # stage so PE's in-order queue never parks on the Act engine.
            psc = None
            pend = []  # queue of (p, k, ets); ctx lags scores/exp by 2
            for p in range(NPAIR):
                new_psc = [psC.tile([128, NPT * (DH + 1)], f32,
                                    name=f"psc{2*p+u}", tag="psC")
                           for u in range(2)]
                if p == 0:
                    psc = new_psc
                for k in range(8):
                    pss = [psS.tile([128, 1024], f32,
                                    name=f"ps_s{2*p+u}_{k}", tag="psS")
                           for u in range(2)]
                    if p == 0 and k < 4:
                        # pair-0's first stages outrun the serialized Pool
                        # repack chain; plain fp8 matmuls (1.0 cyc/row) read
                        # the natural layout directly, same numerics.
                        for u in range(2):
                            rs = u * DH
                            for i in range(2):
                                st = 2 * k + i
                                nc.tensor.matmul(
                                    pss[u][:, i * 512:(i + 1) * 512],
                                    lhsT=kt_sb[0][rs:rs + DH,
                                                  st * 128:(st + 1) * 128],
                                    rhs=qt_sb[0][rs:rs + DH, :],
                                    start=True, stop=True)
                    else:
                        k3 = kt_dr[p].rearrange("q (i s) -> q i s", i=2)
                        q3 = qt_dr[p].rearrange("q (i x) -> q i x", i=2)
                        # u-major so pss[u0] is complete after two matmuls
                        # and its exp can issue while u1's scores run.
                        for u in range(2):
                            for i in range(2):
                                st = 2 * k + i
                                nc.tensor.matmul(
                                    pss[u][:, i * 512:(i + 1) * 512],
                                    lhsT=k3[u * 32:(u + 1) * 32, :,
                                            st * 128:(st + 1) * 128],
                                    rhs=q3[u * 32:(u + 1) * 32, :, :],
                                    start=True, stop=True, perf_mode=DR)
                    ets = []
                    for u in range(2):
                        et = expool.tile([128, 1024], bf16,
                                         name=f"et{2*p+u}_{k}", tag="et")
                        nc.scalar.activation(et, pss[u], Exp,
                                             bias=0.0, scale=0.125)
                        ets.append(et)

                    # ---- PE filler for this stage ----
                    if p == 0:
                        if k == 0:
                            emit_qt_repack(1)
                        elif k == 3:
                            emit_kt_sc(0, 2)
                        elif k == 4:
                            emit_kt_sc(1, 0)
                        elif k == 5:
                            emit_kt_sc(0, 3)
                        elif k == 6:
                            emit_kt_sc(1, 1)
                            emit_qt_repack(2)
                        elif k == 7:
                            emit_kt_sc(1, 2)
                            emit_kt_sc(1, 3)
                        ensure_v(2 * k + 1)
                    elif p < NPAIR - 1:
                        if k % 2 == 0:
                            emit_kt_sc(p + 1, k // 2)
                        elif k == 5 and p < NPAIR - 2:
                            emit_qt_repack(p + 2)

                    else:
                        # O-partials T2..T6 at k=3..7 (T0/T1 follow the
                        # transposes(6) in the k==2 post-ctx block);
                        # T7 is held back to cover the tail's exp(7,7) wait.
                        if 3 <= k <= 7:
                            emit_opartial(k - 1)

                    # ---- lagged ctx for the previous stage ----
                    if len(pend) == 2:
                        cp, ck, cets = pend.pop(0)
                        emit_ctx(cp, ck, cets)
                        if ck == 7:
                            emit_normalize(cp)
                            psc = new_psc
                    # transposes of the previous pair go one stage after its
                    # normalize so PE is not queued behind the DVE muls.
                    if k == 2 and p >= 1:
                        emit_transposes(p - 1)
                        if p == NPAIR - 1:
                            emit_opartial(0)
                            emit_opartial(1)
                    pend.append((p, k, ets))

            # ---------------- tail ----------------
            # T7 partial covers PE while Act finishes exp(7,7); then pair-7
            # normalize -> per-p-chunk: transpose, copy, d7 matmul with the
            # bf16 partial accumulated in-PSUM via an identity matmul (PE,
            # 213ns, replaces a 658ns DVE add), Act-engine copy to SBUF
            # (Act is idle in the tail), DMA out.
            cp, ck, cets = pend.pop(0)
            emit_ctx(cp, ck, cets)
            # T7's d-loop is split around ctx(7,7)/normalize so PE covers
            # both the exp(7,7) wait and the normalize-DVE wait.
            ps7 = psP.tile([128, 512], f32, name="ps_op7", tag="psP")
            for d in range(4):
                nc.tensor.matmul(
                    ps7, lhsT=cxT[d][:, 3 * 128:(3 + 1) * 128],
                    rhs=ow_sb[d][:, 512:1024],
                    start=(d == 0), stop=False)
            cp, ck, cets = pend.pop(0)
            emit_ctx(cp, ck, cets)
            for d in range(4, NDT - 2):
                nc.tensor.matmul(
                    ps7, lhsT=cxT[d][:, 3 * 128:(3 + 1) * 128],
                    rhs=ow_sb[d][:, 512:1024],
                    start=False, stop=(d == NDT - 3))
            emit_normalize(NPAIR - 1)
            nc.vector.tensor_add(partial[7], ps7, obb_sb[:, 512:1024])
            pp = NPAIR - 1
            for pc in range(NPT):
                pst = psP.tile([128, 128], bf16, name=f"pst{pp}_{pc}",
                               tag="psP")
                nc.tensor.transpose(
                    pst,
                    ctx_nat[:, pc * D + pp * 128:pc * D + (pp + 1) * 128],
                    ident)
                if pc % 2 == 0:
                    nc.vector.tensor_copy(
                        cxT[pp][:, pc * 128:(pc + 1) * 128], pst)
                else:
                    nc.scalar.copy(cxT[pp][:, pc * 128:(pc + 1) * 128], pst)
            for pc in range(NPT):
                ot = opool.tile([128, 1024], bf16, name=f"ot{pc}", tag="ot",
                                bufs=3)
                for jc in range(2):
                    T = pc * 2 + jc
                    tpool, ttag = ((psS, "psS") if jc == 0 else (psC, "psC"))
                    ps = tpool.tile([128, 512], f32, name=f"ps_o7_{T}",
                                    tag=ttag)
                    for d in (NDT - 2, NDT - 1):
                        nc.tensor.matmul(
                            ps, lhsT=cxT[d][:, pc * 128:(pc + 1) * 128],
                            rhs=ow_sb[d][:, jc * 512:(jc + 1) * 512],
                            start=(d == NDT - 2), stop=False)
                    nc.tensor.matmul(ps, lhsT=ident, rhs=partial[T],
                                     start=False, stop=True)
                    if jc == 0:
                        nc.scalar.copy(ot[:, 0:512], ps)
                    else:
                        nc.vector.tensor_copy(ot[:, 512:1024], ps)
                # one DMA per p-chunk halves the serialized HWDGE issues
                nc.sync.dma_start(
                    out=out.ap()[pc * 128:(pc + 1) * 128, :], in_=ot)


def get_nc():
    global _cached_nc
    if _cached_nc is None:
        _cached_nc = _build_nc()
    return _cached_nc


def make_in_maps(ts_features, llm_features, q_w, q_b, k_w, k_b, v_w, v_b,
                 o_w, o_b):
    ts = np.asarray(ts_features, np.float32)
    llm = np.asarray(llm_features, np.float32)
    shared = {
        "qwT": np.ascontiguousarray(np.asarray(q_w, np.float32).T).astype(_BF16),
        "kwT": np.ascontiguousarray(np.asarray(k_w, np.float32).T).astype(_BF16),
        "vwT": np.ascontiguousarray(np.asarray(v_w, np.float32).T).astype(_BF16),
        "owT": np.ascontiguousarray(np.asarray(o_w, np.float32).T).astype(_BF16),
        "qkb": np.ascontiguousarray(np.concatenate(
            [np.asarray(q_b, np.float32).reshape(NDT, 128).T,
             np.asarray(k_b, np.float32).reshape(NDT, 128).T], axis=1)),
        "vbb": np.ascontiguousarray(
            np.broadcast_to(np.asarray(v_b, np.float32), (128, D))).astype(_BF16),
        "obb": np.ascontiguousarray(
            np.broadcast_to(np.asarray(o_b, np.float32), (128, D))).astype(_BF16),
    }
    in_maps = []
    for b in range(NCORES):
        m = dict(shared)
        m["tsT"] = np.ascontiguousarray(ts[b].T).astype(_BF16)
        m["llmT"] = np.ascontiguousarray(llm[b].T).astype(_BF16)
        in_maps.append(m)
    return in_maps


def kernel(**inputs):
    from concourse.bass_utils import run_bass_kernel_spmd

    nc = get_nc()
    in_maps = make_in_maps(**inputs)
    res = run_bass_kernel_spmd(nc, in_maps, list(range(NCORES)))
    return np.stack([res.results[i]["out"] for i in range(NCORES)],
                    axis=0).astype(np.float32)



# revision 8
# speedup vs baseline: 1.2653x; 1.2653x over previous
"""CrossAttention kernel for 8 Trainium2 NeuronCores — v2 (Act-bound).

Reference (per batch element b, one core each):
    q = ts[b] @ q_w.T + q_b; k/v = llm[b] @ {k,v}_w.T + b
    per head h: ctx_h = softmax(q_h k_h^T / 8) v_h;  out = ctx @ o_w.T + o_b

v2 rationale: under the TimelineSim cost model the Act engine's exp
stream is the hard floor (16.8M softmax elements / 128 partitions x
0.83 ns = ~133 us).  v1 was PE-bound at ~197 us; v2 moves the big
projections to fp8e4 DoubleRow (0.5 cyc/row, half the passes) with
residual (hi+lo) splits to keep fp8 quantization error in check:

  QT/KT/V' schemes (contraction 1024 = 4 DR pairs of 256):
    qproj  fp8s: (ts_hi + ts_lo) x qw_hi          8 DR mm / tile
    kproj  fp8s: (llm_hi + llm_lo) x kw_hi        8 DR mm / (jt,sc)
    vproj  fp8s: (llm_hi + llm_lo) x vw_hi        8 DR mm / (st,jc)
  Weights are host-scaled x16 before the fp8 cast (their U(-1/32,1/32)
  range would land in fp8e4m3 denormals); the evacuation fuses the /16
  with the bias add (two-op tensor_scalar).  qt/kt are stored x8 in fp8
  (cuts the cast's denormal tail); the exp scale absorbs the /64.
  Scores stay fp8-DR (qt/kt repacked to [32,2,*]); ctx + O-proj stay
  bf16.  Numpy-simulated end-to-end rel err: 1.55e-2 (gate 2e-2).

Schedule: Act streams 2 exps/stage (2076 ns) for 64 stages; PE supplies
scores just-in-time and fills the rest of each stage with projections.
ctx lags ONE PAIR (8 stages) so V' emission spreads at ~1 tile/stage
over pairs 0-1 instead of 2/tile (halves the early Act starvation).
psc PSUM pair is reused every pair (normalize(p) frees it before
ctx(p+1) starts).  O-partials (d0..5) run in pair 7; the tail does
ctx(7,*), normalize, transposes(6,7) and the d6/d7+partial matmuls.

Input DMAs are spread over four issue queues (sync/scalar/vector 565-
667 ns per issue, gpsimd SWDGE ~1 us gen) in consumption order so the
first exp fires ~6 us in.
"""
import numpy as np
import ml_dtypes

D = 1024          # d_model
P = 512           # ts sequence length
S = 2048          # llm sequence length
H = 16            # heads
DH = 64           # head dim
NCORES = 8
NDT = D // 128    # 8 d-tiles
NDP = 4           # 4 d-pairs (DoubleRow: 256 contraction each)
NST = S // 128    # 16 s-tiles
NPT = P // 128    # 4 p-tiles
NPAIR = H // 2    # 8 head pairs

_BF16 = ml_dtypes.bfloat16
_F8 = ml_dtypes.float8_e4m3fn

_cached_nc = None


def _build_nc():
    import concourse.tile as tile
    from concourse import bacc, mybir

    f32 = mybir.dt.float32
    bf16 = mybir.dt.bfloat16

    nc = bacc.Bacc("TRN2", target_bir_lowering=False, debug=False,
                   num_devices=NCORES)

    f8 = mybir.dt.float8e4
    tsh = nc.declare_dram_parameter("tsh", [D, P], f8, isOutput=False)
    tsl = nc.declare_dram_parameter("tsl", [D, P], f8, isOutput=False)
    llmh = nc.declare_dram_parameter("llmh", [D, S], f8, isOutput=False)
    llml = nc.declare_dram_parameter("llml", [D, S], f8, isOutput=False)
    qwh = nc.declare_dram_parameter("qwh", [D, D], f8, isOutput=False)
    kwh = nc.declare_dram_parameter("kwh", [D, D], f8, isOutput=False)
    vwh = nc.declare_dram_parameter("vwh", [D, D], f8, isOutput=False)
    owT = nc.declare_dram_parameter("owT", [D, D], bf16, isOutput=False)
    qkb = nc.declare_dram_parameter("qkb", [128, 2 * NDT], f32, isOutput=False)
    vbb = nc.declare_dram_parameter("vbb", [128, D], bf16, isOutput=False)
    obb = nc.declare_dram_parameter("obb", [128, D], bf16, isOutput=False)
    out = nc.declare_dram_parameter("out", [P, D], bf16, isOutput=True)

    with tile.TileContext(nc) as tc:
        _emit(tc, nc, tile, mybir, f32, bf16, f8,
              tsh, tsl, llmh, llml, qwh, kwh, vwh, owT, qkb, vbb, obb, out)
    nc.compile()
    return nc


def _emit(tc, nc, tile, mybir, f32, bf16, f8,
          tsh, tsl, llmh, llml, qwh, kwh, vwh, owT, qkb, vbb, obb, out):
    from contextlib import ExitStack
    from concourse.masks import make_identity

    Exp = mybir.ActivationFunctionType.Exp
    DR = mybir.MatmulPerfMode.DoubleRow
    MUL = mybir.AluOpType.mult
    ADD = mybir.AluOpType.add
    EXP_SCALE = 0.125 / 64.0   # scores carry x8 * x8 from the fp8 stores

    with ExitStack() as ctx:
        persist = ctx.enter_context(tc.tile_pool(name="persist", bufs=1))
        ktpool = ctx.enter_context(tc.tile_pool(name="ktpool", bufs=2))
        qtpool = ctx.enter_context(tc.tile_pool(name="qtpool", bufs=3))
        ktdrpool = ctx.enter_context(tc.tile_pool(name="ktdrpool", bufs=3))
        qtdrpool = ctx.enter_context(tc.tile_pool(name="qtdrpool", bufs=3))
        expool = ctx.enter_context(tc.tile_pool(name="expool", bufs=20))
        rpool = ctx.enter_context(tc.tile_pool(name="rpool", bufs=2))
        opool = ctx.enter_context(tc.tile_pool(name="opool", bufs=5))

        ident = persist.tile([128, 128], bf16, name="ident", tag="ident")
        make_identity(nc, ident)

        def dr_tile(name, cols):
            t = persist.tile([128, 2 * cols], f8, name=name, tag=name)
            return t, t.rearrange("p (i c) -> p i c", i=2)

        # ---- persistent input tiles (DoubleRow [128, 2, cols] layouts) ----
        ts_h, ts_h3 = zip(*[dr_tile(f"ts_h{dp}", P) for dp in range(NDP)])
        ts_l, ts_l3 = zip(*[dr_tile(f"ts_l{dp}", P) for dp in range(NDP)])
        llm_h, llm_h3 = zip(*[dr_tile(f"llm_h{dp}", S) for dp in range(NDP)])
        llm_l, llm_l3 = zip(*[dr_tile(f"llm_l{dp}", S) for dp in range(NDP)])
        qw_h, qw_h3 = zip(*[dr_tile(f"qw_h{dp}", D) for dp in range(NDP)])
        kw_h, kw_h3 = zip(*[dr_tile(f"kw_h{dp}", D) for dp in range(NDP)])
        vw_h, vw_h3 = zip(*[dr_tile(f"vw_h{dp}", D) for dp in range(NDP)])
        qkb_sb = persist.tile([128, 2 * NDT], f32, name="qkb_sb", tag="qkb_sb")
        vbb_sb = persist.tile([128, D], bf16, name="vbb_sb", tag="vbb_sb")
        obb_sb = persist.tile([128, D], bf16, name="obb_sb", tag="obb_sb")
        ow_sb = []
        for d in range(NDT):
            ow_sb.append(persist.tile([128, D], bf16, name=f"ow_sb{d}",
                                      tag=f"ow_sb{d}"))

        # ---- input DMAs, four queues, consumption order ----
        def dma_dr(eng, dst3, dram, dp, lo, hi):
            # dst3[:, :, lo:hi] <- dram rows [dp*256 .. dp*256+256), cols lo:hi
            src = dram.ap()[dp * 256:(dp + 1) * 256, lo:hi] \
                .rearrange("(i p) c -> p i c", i=2)
            eng.dma_start(out=dst3[:, :, lo:hi], in_=src)

        # sync (SP) queue: ts + llm_h + small consts, then O weights late
        for dp in range(NDP):
            dma_dr(nc.sync, ts_h3[dp], tsh, dp, 0, P)
        nc.sync.dma_start(out=qkb_sb, in_=qkb.ap())
        for dp in range(NDP):
            dma_dr(nc.sync, llm_h3[dp], llmh, dp, 0, 1024)
        for dp in range(NDP):
            dma_dr(nc.sync, llm_h3[dp], llmh, dp, 1024, 2048)
        nc.sync.dma_start(out=vbb_sb, in_=vbb.ap())
        nc.sync.dma_start(out=obb_sb, in_=obb.ap())
        for d in range(NDT):
            nc.sync.dma_start(out=ow_sb[d],
                              in_=owT.ap()[d * 128:(d + 1) * 128, :])

        # scalar (Act HWDGE) queue: weights in consumption order
        for dp in range(NDP):
            dma_dr(nc.scalar, qw_h3[dp], qwh, dp, 0, 256)
        for dp in range(NDP):
            dma_dr(nc.scalar, kw_h3[dp], kwh, dp, 0, 256)
        for dp in range(NDP):
            dma_dr(nc.scalar, vw_h3[dp], vwh, dp, 0, 1024)
        for dp in range(NDP):
            dma_dr(nc.scalar, qw_h3[dp], qwh, dp, 256, 1024)
        for dp in range(NDP):
            dma_dr(nc.scalar, kw_h3[dp], kwh, dp, 256, 1024)

        # gpsimd (SWDGE) queue: residual-lo activations; llm_l's second
        # half is emitted mid-pair-0 so the qt repacks aren't queued
        # behind it (Pool FIFO).
        for dp in range(NDP):
            dma_dr(nc.gpsimd, ts_l3[dp], tsl, dp, 0, P)
        for dp in range(NDP):
            dma_dr(nc.gpsimd, llm_l3[dp], llml, dp, 0, 1024)

        def dma_llml_b():
            for dp in range(NDP):
                dma_dr(nc.gpsimd, llm_l3[dp], llml, dp, 1024, 2048)

        # ---- on-chip intermediates ----
        qt_sb = [None] * NDT      # [128, P] f8, 8*q values
        qt_dr = [None] * NDT
        kt_sb = [None] * NDT      # [128, S] f8, 8*k values
        kt_dr = [None] * NDT
        vp_sb = [None] * NST      # [128, H*(DH+1)] bf16
        ctx_nat = persist.tile([128, NPT * D], bf16, name="ctx_nat",
                               tag="ctx_nat")
        cxT = []
        for d in range(NDT):
            cxT.append(persist.tile([128, P], bf16, name=f"cxT{d}",
                                    tag=f"cxT{d}"))
        partial = []
        for T in range(8):
            partial.append(persist.tile([128, 512], bf16, name=f"opart{T}",
                                        tag=f"opart{T}"))

        with tc.tile_pool(name="psS", bufs=2, space="PSUM") as psS, \
             tc.tile_pool(name="psC", bufs=2, space="PSUM") as psC, \
             tc.tile_pool(name="psP", bufs=2, space="PSUM") as psP:

            # ---------------- emission helpers ----------------
            def emit_qt(jt):
                # QT[j,p] (x8, fp8): (ts_h + ts_l) x qw_h, DR over d-pairs.
                ps = psP.tile([128, P], f32, name=f"ps_q{jt}", tag="psP")
                qt_sb[jt] = qtpool.tile([128, P], f8, name=f"qt_sb{jt}",
                                        tag="qt")
                n = 2 * NDP
                i = 0
                for rhs3 in (ts_h3, ts_l3):
                    for dp in range(NDP):
                        nc.tensor.matmul(
                            ps,
                            lhsT=qw_h3[dp][:, :, jt * 128:(jt + 1) * 128],
                            rhs=rhs3[dp],
                            start=(i == 0), stop=(i == n - 1), perf_mode=DR)
                        i += 1
                # psum holds 16*q; store 8*q + 8*qb
                nc.vector.tensor_scalar(qt_sb[jt], ps, 0.5,
                                        qkb_sb[:, jt:jt + 1], MUL, ADD)

            def emit_qt_repack(jt):
                qt_dr[jt] = qtdrpool.tile([64, 2 * P], f8,
                                          name=f"qt_dr{jt}", tag="qtdr")
                q3 = qt_dr[jt].rearrange("q (i x) -> q i x", i=2)
                for u in range(2):
                    for i in range(2):
                        nc.gpsimd.dma_start(
                            out=q3[u * 32:(u + 1) * 32, i:i + 1, :],
                            in_=qt_sb[jt][u * 64 + i * 32:
                                          u * 64 + i * 32 + 32, :])

            def emit_kt_sc(jt, sc, lite=False):
                # KT[j,s] (x8, fp8) one 512-col s-chunk; lite skips llm_l.
                if kt_sb[jt] is None:
                    kt_sb[jt] = ktpool.tile([128, S], f8,
                                            name=f"kt_sb{jt}", tag="kt")
                    kt_dr[jt] = ktdrpool.tile([64, 2 * S], f8,
                                              name=f"kt_dr{jt}", tag="ktdr")
                ps = psP.tile([128, 512], f32, name=f"ps_k{jt}_{sc}",
                              tag="psP")
                rhs_sets = (llm_h3,) if lite else (llm_h3, llm_l3)
                n = len(rhs_sets) * NDP
                i = 0
                for rhs3 in rhs_sets:
                    for dp in range(NDP):
                        nc.tensor.matmul(
                            ps,
                            lhsT=kw_h3[dp][:, :, jt * 128:(jt + 1) * 128],
                            rhs=rhs3[dp][:, :, sc * 512:(sc + 1) * 512],
                            start=(i == 0), stop=(i == n - 1), perf_mode=DR)
                        i += 1
                nc.vector.tensor_scalar(
                    kt_sb[jt][:, sc * 512:(sc + 1) * 512], ps, 0.5,
                    qkb_sb[:, NDT + jt:NDT + jt + 1], MUL, ADD)

            def emit_dr_repack(jt, lo, hi):
                k3 = kt_dr[jt].rearrange("q (i s) -> q i s", i=2)
                for u in range(2):
                    for i in range(2):
                        nc.gpsimd.dma_start(
                            out=k3[u * 32:(u + 1) * 32, i:i + 1, lo:hi],
                            in_=kt_sb[jt][u * 64 + i * 32:u * 64 + i * 32 + 32,
                                          lo:hi])

            def emit_v(st):
                # V'[s, h*65+x] bf16; psum = 16*v -> *1/16 + vb on evac.
                vp = persist.tile([128, H * (DH + 1)], bf16,
                                  name=f"vp_sb{st}", tag=f"vp_sb{st}")
                vp3 = vp.rearrange("p (h x) -> p h x", x=DH + 1)
                nc.gpsimd.memset(vp3[:, :, DH:DH + 1], 1.0)
                for jc in range(2):
                    ps = psP.tile([128, 512], f32, name=f"ps_v{st}_{jc}",
                                  tag="psP")
                    n = 2 * NDP
                    i = 0
                    for lhs3 in (llm_h3, llm_l3):
                        for dp in range(NDP):
                            nc.tensor.matmul(
                                ps,
                                lhsT=lhs3[dp][:, :, st * 128:(st + 1) * 128],
                                rhs=vw_h3[dp][:, :, jc * 512:(jc + 1) * 512],
                                start=(i == 0), stop=(i == n - 1),
                                perf_mode=DR)
                            i += 1
                    nc.vector.scalar_tensor_tensor(
                        vp3[:, jc * 8:(jc + 1) * 8, 0:DH],
                        ps.rearrange("p (h x) -> p h x", x=DH),
                        1.0 / 16.0,
                        vbb_sb[:, jc * 512:(jc + 1) * 512]
                        .rearrange("p (h x) -> p h x", x=DH),
                        MUL, ADD)
                vp_sb[st] = vp

            def emit_ctx(p, k, ets):
                # ctx[p-chunk, 0:65] += expT_h(st).T @ V'_h (bf16).
                for i in range(2):
                    st = 2 * k + i
                    for u in range(2):
                        h = 2 * p + u
                        for pc in range(NPT):
                            nc.tensor.matmul(
                                psc[u][:, pc * (DH + 1):
                                       (pc + 1) * (DH + 1)],
                                lhsT=ets[u][:, i * 512 + pc * 128:
                                            i * 512 + (pc + 1) * 128],
                                rhs=vp_sb[st][:, h * (DH + 1):
                                              (h + 1) * (DH + 1)],
                                start=(st == 0 and pc == 0),
                                stop=(st == NST - 1 and pc == NPT - 1))

            def emit_normalize(p, act_split=False):
                rcs = []
                for u in range(2):
                    h = 2 * p + u
                    psc3 = psc[u].rearrange("p (c x) -> p c x", x=DH + 1)
                    rc = rpool.tile([128, NPT], f32, name=f"rc{h}", tag="rc")
                    rc3 = rc.rearrange("p (c x) -> p c x", x=1)
                    nc.vector.reciprocal(rc3, psc3[:, :, DH:DH + 1])
                    rcs.append(rc)
                for pc in range(NPT):
                    for u in range(2):
                        h = 2 * p + u
                        dst = ctx_nat[:, pc * D + h * DH:pc * D + (h + 1) * DH]
                        srcp = psc[u][:, pc * (DH + 1):pc * (DH + 1) + DH]
                        if act_split and u == 1:
                            nc.scalar.mul(dst, srcp, rcs[u][:, pc:pc + 1])
                        else:
                            nc.vector.tensor_scalar_mul(
                                dst, srcp, rcs[u][:, pc:pc + 1])

            def emit_transposes(p, act_split=False):
                for pc in range(NPT):
                    pst = psP.tile([128, 128], bf16, name=f"pst{p}_{pc}",
                                   tag="psP")
                    nc.tensor.transpose(
                        pst,
                        ctx_nat[:, pc * D + p * 128:pc * D + (p + 1) * 128],
                        ident)
                    if act_split and pc % 2 == 1:
                        nc.scalar.copy(cxT[p][:, pc * 128:(pc + 1) * 128],
                                       pst)
                    else:
                        nc.vector.tensor_copy(
                            cxT[p][:, pc * 128:(pc + 1) * 128], pst)

            def emit_opartial(T):
                # out tile T=(pt,jc): bf16 sum d=0..5 plus output bias.
                pt, jc = T // 2, T % 2
                ps = psP.tile([128, 512], f32, name=f"ps_op{T}", tag="psP")
                for d in range(6):
                    nc.tensor.matmul(
                        ps, lhsT=cxT[d][:, pt * 128:(pt + 1) * 128],
                        rhs=ow_sb[d][:, jc * 512:(jc + 1) * 512],
                        start=(d == 0), stop=(d == 5))
                nc.vector.tensor_add(partial[T], ps,
                                     obb_sb[:, jc * 512:(jc + 1) * 512])

            # ---------------- prologue ----------------
            emit_qt(0)
            emit_qt(1)
            emit_qt_repack(0)
            emit_qt_repack(1)
            emit_kt_sc(0, 0, lite=True)
            emit_kt_sc(0, 1, lite=True)

            # ---------------- pipelined head pairs ----------------
            # ctx lags ONE PAIR (8 stages): pend queue of stage records.
            psc = None
            pend = []
            emitted_v = [0]

            def ensure_v(upto):
                while emitted_v[0] <= min(upto, NST - 1):
                    emit_v(emitted_v[0])
                    emitted_v[0] += 1

            for p in range(NPAIR):
                for k in range(8):
                    g = 8 * p + k          # global stage index
                    if psc is None:
                        psc = [psC.tile([128, NPT * (DH + 1)], f32,
                                        name=f"psc{u}", tag="psC")
                               for u in range(2)]
                    pss = [psS.tile([128, 1024], f32,
                                    name=f"ps_s{2*p+u}_{k}", tag="psS")
                           for u in range(2)]
                    if p == 0 and k < 4:
                        # plain fp8 matmuls on the natural layout while the
                        # repack chain catches up
                        for u in range(2):
                            rs = u * DH
                            for i in range(2):
                                st = 2 * k + i
                                nc.tensor.matmul(
                                    pss[u][:, i * 512:(i + 1) * 512],
                                    lhsT=kt_sb[0][rs:rs + DH,
                                                  st * 128:(st + 1) * 128],
                                    rhs=qt_sb[0][rs:rs + DH, :],
                                    start=True, stop=True)
                    else:
                        k3 = kt_dr[p].rearrange("q (i s) -> q i s", i=2)
                        q3 = qt_dr[p].rearrange("q (i x) -> q i x", i=2)
                        for u in range(2):
                            for i in range(2):
                                st = 2 * k + i
                                nc.tensor.matmul(
                                    pss[u][:, i * 512:(i + 1) * 512],
                                    lhsT=k3[u * 32:(u + 1) * 32, :,
                                            st * 128:(st + 1) * 128],
                                    rhs=q3[u * 32:(u + 1) * 32, :, :],
                                    start=True, stop=True, perf_mode=DR)
                    ets = []
                    for u in range(2):
                        et = expool.tile([128, 1024], bf16,
                                         name=f"et{2*p+u}_{k}", tag="et")
                        nc.scalar.activation(et, pss[u], Exp,
                                             bias=0.0, scale=EXP_SCALE)
                        ets.append(et)

                    # ---- PE fillers ----
                    # V': ~1 tile/stage through pairs 0-1 (+2 stage offset,
                    # catch-up at the end of pair 1)
                    if g >= 2:
                        if g < 13:
                            ensure_v(g - 2)
                        elif g == 13:
                            ensure_v(11)
                        elif g == 14:
                            ensure_v(13)
                        elif g == 15:
                            ensure_v(15)
                    # KT(1) in pair 0 (plus pair 0's own sc2/3); KT(p+1)
                    # during pair p for p>=1.  QT(2) late in pair 0;
                    # QT(p+2) at (p,5) + repack at (p,6) for p=1..5.
                    if p == 0:
                        if k == 0:
                            dma_llml_b()
                        elif k == 1:
                            emit_kt_sc(0, 2, lite=True)
                        elif k == 2:
                            emit_kt_sc(0, 3, lite=True)
                            emit_dr_repack(0, 1024, 2048)
                        elif k == 4:
                            emit_kt_sc(1, 0)
                        elif k == 5:
                            emit_kt_sc(1, 1)
                            emit_dr_repack(1, 0, 1024)
                        elif k == 6:
                            emit_kt_sc(1, 2)
                            emit_qt(2)
                        elif k == 7:
                            emit_kt_sc(1, 3)
                            emit_dr_repack(1, 1024, 2048)
                            emit_qt_repack(2)
                    elif p < NPAIR - 1:
                        if k < 4:
                            emit_kt_sc(p + 1, k)
                            if k == 1:
                                emit_dr_repack(p + 1, 0, 1024)
                            elif k == 3:
                                emit_dr_repack(p + 1, 1024, 2048)
                        elif k == 5 and p <= 5:
                            emit_qt(p + 2)
                        elif k == 6 and p <= 5:
                            emit_qt_repack(p + 2)
                    else:
                        # pair 7: O-proj partials T0..T5 (d0..5)
                        if 2 <= k <= 7:
                            emit_opartial(k - 2)
                    # transposes(p-2) once normalize(p-2) has run
                    if k == 1 and p >= 2:
                        emit_transposes(p - 2)

                    # ---- lagged ctx: one pair behind ----
                    pend.append((p, k, ets))
                    if len(pend) > 8:
                        cp, ck, cets = pend.pop(0)
                        emit_ctx(cp, ck, cets)
                        if ck == 7:
                            emit_normalize(cp)
                            psc = None

            # ---------------- tail ----------------
            # ctx(7,*) + T6/T7 partials interleaved, then normalize(7),
            # transposes(6,7), final d6/d7 + partial accumulate, out DMA.
            psc = [psC.tile([128, NPT * (DH + 1)], f32,
                            name=f"psc_t{u}", tag="psC") for u in range(2)]
            for idx in range(4):
                cp, ck, cets = pend.pop(0)
                emit_ctx(cp, ck, cets)
            emit_opartial(6)
            for idx in range(4):
                cp, ck, cets = pend.pop(0)
                emit_ctx(cp, ck, cets)
                if ck == 7:
                    emit_normalize(cp, act_split=True)
            emit_opartial(7)
            emit_transposes(6, act_split=True)
            emit_transposes(7, act_split=True)
            for pc in range(NPT):
                ot = opool.tile([128, 1024], bf16, name=f"ot{pc}", tag="ot",
                                bufs=3)
                for jc in range(2):
                    T = pc * 2 + jc
                    tpool, ttag = ((psS, "psS") if jc == 0 else (psC, "psC"))
                    ps = tpool.tile([128, 512], f32, name=f"ps_o7_{T}",
                                    tag=ttag)
                    for d in (6, 7):
                        nc.tensor.matmul(
                            ps, lhsT=cxT[d][:, pc * 128:(pc + 1) * 128],
                            rhs=ow_sb[d][:, jc * 512:(jc + 1) * 512],
                            start=(d == 6), stop=False)
                    nc.tensor.matmul(ps, lhsT=ident, rhs=partial[T],
                                     start=False, stop=True)
                    if jc == 0:
                        nc.scalar.copy(ot[:, 0:512], ps)
                    else:
                        nc.vector.tensor_copy(ot[:, 512:1024], ps)
                nc.sync.dma_start(
                    out=out.ap()[pc * 128:(pc + 1) * 128, :], in_=ot)


def get_nc():
    global _cached_nc
    if _cached_nc is None:
        _cached_nc = _build_nc()
    return _cached_nc


def _split8(x):
    hi = x.astype(_F8)
    lo = (x - hi.astype(np.float32)).astype(_F8)
    return hi, lo


def make_in_maps(ts_features, llm_features, q_w, q_b, k_w, k_b, v_w, v_b,
                 o_w, o_b):
    ts = np.asarray(ts_features, np.float32)
    llm = np.asarray(llm_features, np.float32)
    qwT = np.ascontiguousarray(np.asarray(q_w, np.float32).T)
    kwT = np.ascontiguousarray(np.asarray(k_w, np.float32).T)
    vwT = np.ascontiguousarray(np.asarray(v_w, np.float32).T)
    owT = np.ascontiguousarray(np.asarray(o_w, np.float32).T)
    shared = {
        "qwh": np.ascontiguousarray((16.0 * qwT).astype(_F8)),
        "kwh": np.ascontiguousarray((16.0 * kwT).astype(_F8)),
        "vwh": np.ascontiguousarray((16.0 * vwT).astype(_F8)),
        "owT": owT.astype(_BF16),
        # biases for the x8-scaled fp8 qt/kt stores
        "qkb": np.ascontiguousarray(np.concatenate(
            [8.0 * np.asarray(q_b, np.float32).reshape(NDT, 128).T,
             8.0 * np.asarray(k_b, np.float32).reshape(NDT, 128).T], axis=1)),
        "vbb": np.ascontiguousarray(
            np.broadcast_to(np.asarray(v_b, np.float32), (128, D))).astype(_BF16),
        "obb": np.ascontiguousarray(
            np.broadcast_to(np.asarray(o_b, np.float32), (128, D))).astype(_BF16),
    }
    in_maps = []
    for b in range(NCORES):
        m = dict(shared)
        tsT = np.ascontiguousarray(ts[b].T)
        llmT = np.ascontiguousarray(llm[b].T)
        m["tsh"], m["tsl"] = _split8(tsT)
        m["llmh"], m["llml"] = _split8(llmT)
        in_maps.append(m)
    return in_maps


def kernel(**inputs):
    from concourse.bass_utils import run_bass_kernel_spmd

    nc = get_nc()
    in_maps = make_in_maps(**inputs)
    res = run_bass_kernel_spmd(nc, in_maps, list(range(NCORES)))
    return np.stack([res.results[i]["out"] for i in range(NCORES)],
                    axis=0).astype(np.float32)


# revision 32
# speedup vs baseline: 1.3882x; 1.0972x over previous
"""CrossAttention kernel for 8 Trainium2 NeuronCores — v2 (Act-bound).

Reference (per batch element b, one core each):
    q = ts[b] @ q_w.T + q_b; k/v = llm[b] @ {k,v}_w.T + b
    per head h: ctx_h = softmax(q_h k_h^T / 8) v_h;  out = ctx @ o_w.T + o_b

v2 rationale: under the TimelineSim cost model the Act engine's exp
stream is the hard floor (16.8M softmax elements / 128 partitions x
0.83 ns = ~133 us).  v1 was PE-bound at ~197 us; v2 moves the big
projections to fp8e4 DoubleRow (0.5 cyc/row, half the passes) with
residual (hi+lo) splits to keep fp8 quantization error in check:

  QT/KT/V' schemes (contraction 1024 = 4 DR pairs of 256):
    qproj  fp8s: (ts_hi + ts_lo) x qw_hi          8 DR mm / tile
    kproj  fp8s: (llm_hi + llm_lo) x kw_hi        8 DR mm / (jt,sc)
    vproj  fp8s: (llm_hi + llm_lo) x vw_hi        8 DR mm / (st,jc)
  Weights are host-scaled x16 before the fp8 cast (their U(-1/32,1/32)
  range would land in fp8e4m3 denormals); the evacuation fuses the /16
  with the bias add (two-op tensor_scalar).  qt/kt are stored x8 in fp8
  (cuts the cast's denormal tail); the exp scale absorbs the /64.
  Scores stay fp8-DR (qt/kt repacked to [32,2,*]); ctx + O-proj stay
  bf16.  Numpy-simulated end-to-end rel err: 1.55e-2 (gate 2e-2).

Schedule: Act streams 2 exps/stage (2076 ns) for 64 stages; PE supplies
scores just-in-time and fills the rest of each stage with projections.
ctx lags ONE PAIR (8 stages) so V' emission spreads at ~1 tile/stage
over pairs 0-1 instead of 2/tile (halves the early Act starvation).
psc PSUM pair is reused every pair (normalize(p) frees it before
ctx(p+1) starts).  O-partials (d0..5) run in pair 7; the tail does
ctx(7,*), normalize, transposes(6,7) and the d6/d7+partial matmuls.

Input DMAs are spread over four issue queues (sync/scalar/vector 565-
667 ns per issue, gpsimd SWDGE ~1 us gen) in consumption order so the
first exp fires ~6 us in.
"""
import numpy as np
import ml_dtypes

D = 1024          # d_model
P = 512           # ts sequence length
S = 2048          # llm sequence length
H = 16            # heads
DH = 64           # head dim
NCORES = 8
NDT = D // 128    # 8 d-tiles
NDP = 4           # 4 d-pairs (DoubleRow: 256 contraction each)
NST = S // 128    # 16 s-tiles
NPT = P // 128    # 4 p-tiles
NPAIR = H // 2    # 8 head pairs

_BF16 = ml_dtypes.bfloat16
_F8 = ml_dtypes.float8_e4m3fn

_cached_nc = None


def _build_nc():
    import concourse.tile as tile
    from concourse import bacc, mybir

    f32 = mybir.dt.float32
    bf16 = mybir.dt.bfloat16

    nc = bacc.Bacc("TRN2", target_bir_lowering=False, debug=False,
                   num_devices=NCORES)

    f8 = mybir.dt.float8e4
    tsh = nc.declare_dram_parameter("tsh", [D, P], f8, isOutput=False)
    tsl = nc.declare_dram_parameter("tsl", [D, P], f8, isOutput=False)
    llmh = nc.declare_dram_parameter("llmh", [D, S], f8, isOutput=False)
    llml = nc.declare_dram_parameter("llml", [D, S], f8, isOutput=False)
    # combined q/k weights, columns [qj01|kj01|qrest|krest] (permuted)
    qkw = nc.declare_dram_parameter("qkw", [D, 2 * D], f8, isOutput=False)
    vwh = nc.declare_dram_parameter("vwh", [D, D], f8, isOutput=False)
    owT = nc.declare_dram_parameter("owT", [D, D], bf16, isOutput=False)
    qkb = nc.declare_dram_parameter("qkb", [128, 2 * NDT], f32, isOutput=False)
    vbb = nc.declare_dram_parameter("vbb", [128, D], bf16, isOutput=False)
    obb = nc.declare_dram_parameter("obb", [128, D], bf16, isOutput=False)
    out = nc.declare_dram_parameter("out", [P, D], bf16, isOutput=True)

    with tile.TileContext(nc) as tc:
        _emit(tc, nc, tile, mybir, f32, bf16, f8,
              tsh, tsl, llmh, llml, qkw, vwh, owT, qkb, vbb, obb, out)
    nc.compile()
    return nc


def _emit(tc, nc, tile, mybir, f32, bf16, f8,
          tsh, tsl, llmh, llml, qkw, vwh, owT, qkb, vbb, obb, out):
    from contextlib import ExitStack
    from concourse.masks import make_identity

    Exp = mybir.ActivationFunctionType.Exp
    DR = mybir.MatmulPerfMode.DoubleRow
    MUL = mybir.AluOpType.mult
    ADD = mybir.AluOpType.add
    EXP_SCALE = 0.125 / 64.0   # scores carry x8 * x8 from the fp8 stores

    with ExitStack() as ctx:
        persist = ctx.enter_context(tc.tile_pool(name="persist", bufs=1))
        ktpool = ctx.enter_context(tc.tile_pool(name="ktpool", bufs=3))
        qtpool = ctx.enter_context(tc.tile_pool(name="qtpool", bufs=3))
        expool = ctx.enter_context(tc.tile_pool(name="expool", bufs=20))
        rpool = ctx.enter_context(tc.tile_pool(name="rpool", bufs=2))
        opool = ctx.enter_context(tc.tile_pool(name="opool", bufs=5))

        ident = persist.tile([128, 128], bf16, name="ident", tag="ident")
        make_identity(nc, ident)

        # ---- persistent input tiles: ONE tile per tensor, DoubleRow view
        # [128, 8=(dp i), cols].  Element (p, 2dp+i, c) <- dram row
        # dp*256 + i*128 + p, col c.
        def big_tile(name, cols, dt=f8):
            t = persist.tile([128, 8 * cols], dt, name=name, tag=name)
            return t.rearrange("p (g c) -> p g c", g=8)

        ts_h3 = big_tile("ts_h", P)
        ts_l3 = big_tile("ts_l", P)
        llm_h3 = big_tile("llm_h", S)
        llm_l3 = big_tile("llm_l", S)
        qkw3 = big_tile("qkw", 2 * D)
        vw_h3 = big_tile("vw_h", D)
        qkb_sb = persist.tile([128, 2 * NDT], f32, name="qkb_sb", tag="qkb_sb")
        vbb_sb = persist.tile([128, D], bf16, name="vbb_sb", tag="vbb_sb")
        obb_sb = persist.tile([128, D], bf16, name="obb_sb", tag="obb_sb")
        # O weights: one tile, slice [:, d*1024 + jc*512 : ...]
        ow_flat = persist.tile([128, NDT * D], bf16, name="ow_sb", tag="ow_sb")

        def ow_sl(d, jc):
            return ow_flat[:, d * D + jc * 512:d * D + (jc + 1) * 512]

        # combined q/k weight column offsets within the 2048-col inner dim
        def qoff(b):
            return b * 128 if b < 2 else 512 + (b - 2) * 128

        def koff(b):
            return 256 + b * 128 if b < 2 else 1280 + (b - 2) * 128

        # ---- input DMAs: one big transfer each, sync queue only (the Act
        # queue must stay clear of DMA issues or they'd delay the exps;
        # HWDGE is a single shared device anyway).  Consumption order.
        def dma_big(dst3, dram, lo, hi):
            src = dram.ap()[:, lo:hi].rearrange("(g p) c -> p g c", g=8)
            nc.sync.dma_start(out=dst3[:, :, lo:hi], in_=src)

        dma_big(ts_h3, tsh, 0, P)
        dma_big(qkw3, qkw, 0, 512)          # q-j01 + k-j01 blocks
        dma_big(ts_l3, tsl, 0, P)
        dma_big(llm_h3, llmh, 0, 512)
        nc.sync.dma_start(out=qkb_sb, in_=qkb.ap())
        dma_big(vw_h3, vwh, 0, D)
        dma_big(llm_l3, llml, 0, 1024)
        dma_big(llm_h3, llmh, 512, 1024)
        dma_big(qkw3, qkw, 512, 2048)       # q-rest + k-rest
        dma_big(llm_h3, llmh, 1024, 2048)
        dma_big(llm_l3, llml, 1024, 2048)
        nc.sync.dma_start(out=vbb_sb, in_=vbb.ap())
        nc.sync.dma_start(out=obb_sb, in_=obb.ap())
        ow4 = ow_flat.rearrange("p (d j) -> p d j", j=D)
        for half in range(2):
            src = owT.ap()[half * 512:(half + 1) * 512, :] \
                .rearrange("(d p) j -> p d j", d=4)
            nc.sync.dma_start(out=ow4[:, half * 4:(half + 1) * 4, :], in_=src)

        # ---- on-chip intermediates ----
        # qt/kt land DIRECTLY in DoubleRow layout: the host permutes the
        # q/k weight columns so output partition q = jtsub*64+u*32+pr of
        # block (J,i) is head-dim j = (2J+jtsub)*128 + u*64 + i*32 + pr.
        # qt_dr[J] is [128, 2, P] (i-major halves), ktdr likewise over S.
        qt_dr = [None] * NDP      # [128, 2*P] f8, 8*q values
        kt_dr = [None] * (NDP + 1)  # [+1: full-precision redo of J0]
        vp_sb = [None] * NST      # [128, H*(DH+1)] bf16
        ctx_nat = persist.tile([128, NPT * D], bf16, name="ctx_nat",
                               tag="ctx_nat")
        cxT = []
        for d in range(NDT):
            cxT.append(persist.tile([128, P], bf16, name=f"cxT{d}",
                                    tag=f"cxT{d}"))
        partial = []
        for T in range(8):
            partial.append(persist.tile([128, 512], bf16, name=f"opart{T}",
                                        tag=f"opart{T}"))

        with tc.tile_pool(name="psS", bufs=2, space="PSUM") as psS, \
             tc.tile_pool(name="psC", bufs=2, space="PSUM") as psC, \
             tc.tile_pool(name="psP", bufs=2, space="PSUM") as psP:

            # ---------------- emission helpers ----------------
            def emit_qt(J, i):
                # qt block (J,i) -> qt_dr[J] half i.  (ts_h+ts_l) x qw_h.
                if qt_dr[J] is None:
                    qt_dr[J] = qtpool.tile([128, 2 * P], f8,
                                           name=f"qt_dr{J}", tag="qt")
                b = 2 * J + i
                ps = psP.tile([128, P], f32, name=f"ps_q{b}", tag="psP")
                n = 2 * NDP
                g = 0
                for rhs3 in (ts_h3, ts_l3):
                    for dp in range(NDP):
                        nc.tensor.matmul(
                            ps,
                            lhsT=qkw3[:, 2 * dp:2 * dp + 2,
                                      qoff(b):qoff(b) + 128],
                            rhs=rhs3[:, 2 * dp:2 * dp + 2, :],
                            start=(g == 0), stop=(g == n - 1), perf_mode=DR)
                        g += 1
                # psum holds 16*q; store 8*q + 8*qb
                nc.vector.tensor_scalar(qt_dr[J][:, i * P:(i + 1) * P], ps,
                                        0.5, qkb_sb[:, b:b + 1], MUL, ADD)

            def emit_kt_sc(slot, J, i, sc, lite=False):
                # kt block (J,i) s-chunk sc -> kt_dr[slot] half i.
                if kt_dr[slot] is None:
                    kt_dr[slot] = ktpool.tile([128, 2 * S], f8,
                                              name=f"kt_dr{slot}", tag="kt")
                b = 2 * J + i
                ps = psP.tile([128, 512], f32, name=f"ps_k{slot}_{b}_{sc}",
                              tag="psP")
                rhs_sets = (llm_h3,) if lite else (llm_h3, llm_l3)
                n = len(rhs_sets) * NDP
                g = 0
                for rhs3 in rhs_sets:
                    for dp in range(NDP):
                        nc.tensor.matmul(
                            ps,
                            lhsT=qkw3[:, 2 * dp:2 * dp + 2,
                                      koff(b):koff(b) + 128],
                            rhs=rhs3[:, 2 * dp:2 * dp + 2,
                                     sc * 512:(sc + 1) * 512],
                            start=(g == 0), stop=(g == n - 1), perf_mode=DR)
                        g += 1
                nc.vector.tensor_scalar(
                    kt_dr[slot][:, i * S + sc * 512:i * S + (sc + 1) * 512],
                    ps, 0.5, qkb_sb[:, NDT + b:NDT + b + 1], MUL, ADD)

            def emit_v(st, jc):
                # V'[s, h*65+x] bf16, heads jc*8..jc*8+8 only.  ctx for
                # pairs 0-3 reads just the jc=0 half, so jc=1 is deferred
                # to pairs 2-4.  psum = 16*v -> *1/16 + vb on evac.
                if vp_sb[st] is None:
                    vp_sb[st] = persist.tile([128, H * (DH + 1)], bf16,
                                             name=f"vp_sb{st}",
                                             tag=f"vp_sb{st}")
                vp3 = vp_sb[st].rearrange("p (h x) -> p h x", x=DH + 1)
                nc.gpsimd.memset(vp3[:, jc * 8:(jc + 1) * 8, DH:DH + 1], 1.0)
                ps = psP.tile([128, 512], f32, name=f"ps_v{st}_{jc}",
                              tag="psP")
                n = 2 * NDP
                i = 0
                for lhs3 in (llm_h3, llm_l3):
                    for dp in range(NDP):
                        nc.tensor.matmul(
                            ps,
                            lhsT=lhs3[:, 2 * dp:2 * dp + 2,
                                      st * 128:(st + 1) * 128],
                            rhs=vw_h3[:, 2 * dp:2 * dp + 2,
                                      jc * 512:(jc + 1) * 512],
                            start=(i == 0), stop=(i == n - 1),
                            perf_mode=DR)
                        i += 1
                nc.vector.scalar_tensor_tensor(
                    vp3[:, jc * 8:(jc + 1) * 8, 0:DH],
                    ps.rearrange("p (h x) -> p h x", x=DH),
                    1.0 / 16.0,
                    vbb_sb[:, jc * 512:(jc + 1) * 512]
                    .rearrange("p (h x) -> p h x", x=DH),
                    MUL, ADD)

            def emit_ctx(p, k, ets):
                # ctx[p-chunk, 0:65] += expT_h(st).T @ V'_h (bf16).
                for i in range(2):
                    st = 2 * k + i
                    for u in range(2):
                        h = 2 * p + u
                        for pc in range(NPT):
                            nc.tensor.matmul(
                                psc[u][:, pc * (DH + 1):
                                       (pc + 1) * (DH + 1)],
                                lhsT=ets[u][:, i * 512 + pc * 128:
                                            i * 512 + (pc + 1) * 128],
                                rhs=vp_sb[st][:, h * (DH + 1):
                                              (h + 1) * (DH + 1)],
                                start=(st == 0 and pc == 0),
                                stop=(st == NST - 1 and pc == NPT - 1))

            def emit_normalize(p, act_split=False):
                rcs = []
                for u in range(2):
                    h = 2 * p + u
                    psc3 = psc[u].rearrange("p (c x) -> p c x", x=DH + 1)
                    rc = rpool.tile([128, NPT], f32, name=f"rc{h}", tag="rc")
                    rc3 = rc.rearrange("p (c x) -> p c x", x=1)
                    nc.vector.reciprocal(rc3, psc3[:, :, DH:DH + 1])
                    rcs.append(rc)
                for pc in range(NPT):
                    for u in range(2):
                        h = 2 * p + u
                        dst = ctx_nat[:, pc * D + h * DH:pc * D + (h + 1) * DH]
                        srcp = psc[u][:, pc * (DH + 1):pc * (DH + 1) + DH]
                        if act_split and u == 1:
                            nc.scalar.mul(dst, srcp, rcs[u][:, pc:pc + 1])
                        else:
                            nc.vector.tensor_scalar_mul(
                                dst, srcp, rcs[u][:, pc:pc + 1])

            def emit_transposes(p, act_split=False):
                for pc in range(NPT):
                    pst = psP.tile([128, 128], bf16, name=f"pst{p}_{pc}",
                                   tag="psP")
                    nc.tensor.transpose(
                        pst,
                        ctx_nat[:, pc * D + p * 128:pc * D + (p + 1) * 128],
                        ident)
                    if act_split and pc % 2 == 1:
                        nc.scalar.copy(cxT[p][:, pc * 128:(pc + 1) * 128],
                                       pst)
                    else:
                        nc.vector.tensor_copy(
                            cxT[p][:, pc * 128:(pc + 1) * 128], pst)

            def emit_opartial(T):
                # out tile T=(pt,jc): bf16 sum d=0..5 plus output bias.
                pt, jc = T // 2, T % 2
                ps = psP.tile([128, 512], f32, name=f"ps_op{T}", tag="psP")
                for d in range(6):
                    nc.tensor.matmul(
                        ps, lhsT=cxT[d][:, pt * 128:(pt + 1) * 128],
                        rhs=ow_sl(d, jc),
                        start=(d == 0), stop=(d == 5))
                nc.vector.tensor_add(partial[T], ps,
                                     obb_sb[:, jc * 512:(jc + 1) * 512])

            # ---------------- prologue ----------------
            emit_qt(0, 0)
            emit_qt(0, 1)
            emit_kt_sc(0, 0, 0, 0, lite=True)
            emit_kt_sc(0, 0, 1, 0, lite=True)

            # ---- filler slot table: global stage -> emissions.  Balances
            # PE work against the 2.08us/stage Act budget; every entry is
            # placed >= ~2 stages before first consumption.
            sched = {}

            def at(g, fn, *args):
                sched.setdefault(g, []).append((fn, args))

            for j in range(12):               # V' heads 0-7: pairs 0-1
                at(j + 2, emit_v, j, 0)
            for j in (12, 13):
                at(14, emit_v, j, 0)
            for j in (14, 15):
                at(15, emit_v, j, 0)
            for j in range(16):               # V' heads 8-15: pairs 2-4
                at(16 + (11 * j) // 8, emit_v, j, 1)
            for s in (1, 2, 3):               # kt J0-lite rest (pair 0)
                at(2 * (s - 1), emit_kt_sc, 0, 0, 0, s, True)
                at(2 * (s - 1), emit_kt_sc, 0, 0, 1, s, True)
            for Jn in (1, 2, 3):              # kt J1..J3, JIT
                for s in range(4):
                    for i in range(2):
                        at(16 * Jn - 3 + 2 * s + i, emit_kt_sc,
                           Jn, Jn, i, s)
            at(5, emit_qt, 1, 0)              # qt J1..J3
            at(6, emit_qt, 1, 1)
            at(25, emit_qt, 2, 0)
            at(27, emit_qt, 2, 1)
            at(41, emit_qt, 3, 0)
            at(43, emit_qt, 3, 1)
            for T in range(6):                # O partials T0-5 in pair 7
                at(58 + T, emit_opartial, T)

            # ---------------- pipelined head pairs ----------------
            # ctx lags ONE PAIR (8 stages): pend queue of stage records.
            psc = None
            pend = []

            for p in range(NPAIR):
                for k in range(8):
                    g = 8 * p + k          # global stage index
                    if psc is None:
                        psc = [psC.tile([128, NPT * (DH + 1)], f32,
                                        name=f"psc{u}", tag="psC")
                               for u in range(2)]
                    pss = [psS.tile([128, 1024], f32,
                                    name=f"ps_s{2*p+u}_{k}", tag="psS")
                           for u in range(2)]
                    J, jsub = p // 2, p % 2
                    k3 = kt_dr[J].rearrange("q (i s) -> q i s", i=2)
                    q3 = qt_dr[J].rearrange("q (i x) -> q i x", i=2)
                    for u in range(2):
                        base = jsub * 64 + u * 32
                        for i in range(2):
                            st = 2 * k + i
                            nc.tensor.matmul(
                                pss[u][:, i * 512:(i + 1) * 512],
                                lhsT=k3[base:base + 32, :,
                                        st * 128:(st + 1) * 128],
                                rhs=q3[base:base + 32, :, :],
                                start=True, stop=True, perf_mode=DR,
                                tile_position=(base, 0))
                    ets = []
                    for u in range(2):
                        et = expool.tile([128, 1024], bf16,
                                         name=f"et{2*p+u}_{k}", tag="et")
                        nc.scalar.activation(et, pss[u], Exp,
                                             bias=0.0, scale=EXP_SCALE)
                        ets.append(et)

                    # ---- PE fillers from the slot table ----
                    for fn, args in sched.pop(g, ()):
                        fn(*args)
                    # transposes(p-2) once normalize(p-2) has run
                    if k == 1 and p >= 2:
                        emit_transposes(p - 2)

                    # ---- lagged ctx: one pair behind ----
                    pend.append((p, k, ets))
                    if len(pend) > 8:
                        cp, ck, cets = pend.pop(0)
                        emit_ctx(cp, ck, cets)
                        if ck == 7:
                            emit_normalize(cp)
                            psc = None

            # ---------------- tail ----------------
            # ctx(7,*) + T6/T7 partials interleaved, then normalize(7),
            # transposes(6,7), final d6/d7 + partial accumulate, out DMA.
            psc = [psC.tile([128, NPT * (DH + 1)], f32,
                            name=f"psc_t{u}", tag="psC") for u in range(2)]
            for idx in range(4):
                cp, ck, cets = pend.pop(0)
                emit_ctx(cp, ck, cets)
            emit_opartial(6)
            for idx in range(4):
                cp, ck, cets = pend.pop(0)
                emit_ctx(cp, ck, cets)
                if ck == 7:
                    emit_normalize(cp, act_split=True)
            emit_opartial(7)
            emit_transposes(6, act_split=True)
            emit_transposes(7, act_split=True)
            for pc in range(NPT):
                ot = opool.tile([128, 1024], bf16, name=f"ot{pc}", tag="ot",
                                bufs=3)
                for jc in range(2):
                    T = pc * 2 + jc
                    tpool, ttag = ((psS, "psS") if jc == 0 else (psC, "psC"))
                    ps = tpool.tile([128, 512], f32, name=f"ps_o7_{T}",
                                    tag=ttag)
                    for d in (6, 7):
                        nc.tensor.matmul(
                            ps, lhsT=cxT[d][:, pc * 128:(pc + 1) * 128],
                            rhs=ow_sl(d, jc),
                            start=(d == 6), stop=False)
                    nc.tensor.matmul(ps, lhsT=ident, rhs=partial[T],
                                     start=False, stop=True)
                    if jc == 0:
                        nc.scalar.copy(ot[:, 0:512], ps)
                    else:
                        nc.vector.tensor_copy(ot[:, 512:1024], ps)
                nc.sync.dma_start(
                    out=out.ap()[pc * 128:(pc + 1) * 128, :], in_=ot)


def get_nc():
    global _cached_nc
    if _cached_nc is None:
        _cached_nc = _build_nc()
    return _cached_nc


def _split8(x):
    hi = x.astype(_F8)
    lo = (x - hi.astype(np.float32)).astype(_F8)
    return hi, lo


def _dr_perm():
    """Column permutation for q/k weights: new column b*128+q holds
    original head-dim j so the projection lands in DoubleRow layout.
    b = 2J+i, q = jtsub*64 + u*32 + pr -> j = (2J+jtsub)*128+u*64+i*32+pr.
    """
    c = np.arange(D)
    J, i, q = c // 256, (c % 256) // 128, c % 128
    jtsub, u, pr = q // 64, (q % 64) // 32, q % 32
    return (2 * J + jtsub) * 128 + u * 64 + i * 32 + pr


def make_in_maps(ts_features, llm_features, q_w, q_b, k_w, k_b, v_w, v_b,
                 o_w, o_b):
    ts = np.asarray(ts_features, np.float32)
    llm = np.asarray(llm_features, np.float32)
    qwT = np.ascontiguousarray(np.asarray(q_w, np.float32).T)
    kwT = np.ascontiguousarray(np.asarray(k_w, np.float32).T)
    vwT = np.ascontiguousarray(np.asarray(v_w, np.float32).T)
    owT = np.ascontiguousarray(np.asarray(o_w, np.float32).T)
    jmap = _dr_perm()
    qwP = (16.0 * qwT[:, jmap]).astype(_F8)
    kwP = (16.0 * kwT[:, jmap]).astype(_F8)
    # combined layout: [q-j01(256) | k-j01(256) | q-rest(768) | k-rest(768)]
    qkw = np.concatenate(
        [qwP[:, 0:256], kwP[:, 0:256], qwP[:, 256:1024], kwP[:, 256:1024]],
        axis=1)
    shared = {
        "qkw": np.ascontiguousarray(qkw),
        "vwh": np.ascontiguousarray((16.0 * vwT).astype(_F8)),
        "owT": owT.astype(_BF16),
        # biases for the x8-scaled fp8 qt/kt stores, in permuted order
        "qkb": np.ascontiguousarray(np.concatenate(
            [8.0 * np.asarray(q_b, np.float32)[jmap].reshape(NDT, 128).T,
             8.0 * np.asarray(k_b, np.float32)[jmap].reshape(NDT, 128).T],
            axis=1)),
        "vbb": np.ascontiguousarray(
            np.broadcast_to(np.asarray(v_b, np.float32), (128, D))).astype(_BF16),
        "obb": np.ascontiguousarray(
            np.broadcast_to(np.asarray(o_b, np.float32), (128, D))).astype(_BF16),
    }
    in_maps = []
    for b in range(NCORES):
        m = dict(shared)
        tsT = np.ascontiguousarray(ts[b].T)
        llmT = np.ascontiguousarray(llm[b].T)
        m["tsh"], m["tsl"] = _split8(tsT)
        m["llmh"], m["llml"] = _split8(llmT)
        in_maps.append(m)
    return in_maps


def kernel(**inputs):
    from concourse.bass_utils import run_bass_kernel_spmd

    nc = get_nc()
    in_maps = make_in_maps(**inputs)
    res = run_bass_kernel_spmd(nc, in_maps, list(range(NCORES)))
    return np.stack([res.results[i]["out"] for i in range(NCORES)],
                    axis=0).astype(np.float32)


# revision 50
# speedup vs baseline: 1.3912x; 1.0021x over previous
"""CrossAttention kernel for 8 Trainium2 NeuronCores — v2 (Act-bound).

Reference (per batch element b, one core each):
    q = ts[b] @ q_w.T + q_b; k/v = llm[b] @ {k,v}_w.T + b
    per head h: ctx_h = softmax(q_h k_h^T / 8) v_h;  out = ctx @ o_w.T + o_b

v2 rationale: under the TimelineSim cost model the Act engine's exp
stream is the hard floor (16.8M softmax elements / 128 partitions x
0.83 ns = ~133 us).  v1 was PE-bound at ~197 us; v2 moves the big
projections to fp8e4 DoubleRow (0.5 cyc/row, half the passes) with
residual (hi+lo) splits to keep fp8 quantization error in check:

  QT/KT/V' schemes (contraction 1024 = 4 DR pairs of 256):
    qproj  fp8s: (ts_hi + ts_lo) x qw_hi          8 DR mm / tile
    kproj  fp8s: (llm_hi + llm_lo) x kw_hi        8 DR mm / (jt,sc)
    vproj  fp8s: (llm_hi + llm_lo) x vw_hi        8 DR mm / (st,jc)
  Weights are host-scaled x16 before the fp8 cast (their U(-1/32,1/32)
  range would land in fp8e4m3 denormals); the evacuation fuses the /16
  with the bias add (two-op tensor_scalar).  qt/kt are stored x8 in fp8
  (cuts the cast's denormal tail); the exp scale absorbs the /64.
  Scores stay fp8-DR (qt/kt repacked to [32,2,*]); ctx + O-proj stay
  bf16.  Numpy-simulated end-to-end rel err: 1.55e-2 (gate 2e-2).

Schedule: Act streams 2 exps/stage (2076 ns) for 64 stages; PE supplies
scores just-in-time and fills the rest of each stage with projections.
ctx lags ONE PAIR (8 stages) so V' emission spreads at ~1 tile/stage
over pairs 0-1 instead of 2/tile (halves the early Act starvation).
psc PSUM pair is reused every pair (normalize(p) frees it before
ctx(p+1) starts).  O-partials (d0..5) run in pair 7; the tail does
ctx(7,*), normalize, transposes(6,7) and the d6/d7+partial matmuls.

Input DMAs are spread over four issue queues (sync/scalar/vector 565-
667 ns per issue, gpsimd SWDGE ~1 us gen) in consumption order so the
first exp fires ~6 us in.
"""
import numpy as np
import ml_dtypes

D = 1024          # d_model
P = 512           # ts sequence length
S = 2048          # llm sequence length
H = 16            # heads
DH = 64           # head dim
NCORES = 8
NDT = D // 128    # 8 d-tiles
NDP = 4           # 4 d-pairs (DoubleRow: 256 contraction each)
NST = S // 128    # 16 s-tiles
NPT = P // 128    # 4 p-tiles
NPAIR = H // 2    # 8 head pairs

_BF16 = ml_dtypes.bfloat16
_F8 = ml_dtypes.float8_e4m3fn

_cached_nc = None


def _build_nc():
    import concourse.tile as tile
    from concourse import bacc, mybir

    f32 = mybir.dt.float32
    bf16 = mybir.dt.bfloat16

    nc = bacc.Bacc("TRN2", target_bir_lowering=False, debug=False,
                   num_devices=NCORES)

    f8 = mybir.dt.float8e4
    tsh = nc.declare_dram_parameter("tsh", [D, P], f8, isOutput=False)
    tsl = nc.declare_dram_parameter("tsl", [D, P], f8, isOutput=False)
    llmh = nc.declare_dram_parameter("llmh", [D, S], f8, isOutput=False)
    llml = nc.declare_dram_parameter("llml", [D, S], f8, isOutput=False)
    # combined q/k weights, columns [qj01|kj01|qrest|krest] (permuted)
    qkw = nc.declare_dram_parameter("qkw", [D, 2 * D], f8, isOutput=False)
    vwh = nc.declare_dram_parameter("vwh", [D, D], f8, isOutput=False)
    owT = nc.declare_dram_parameter("owT", [D, D], bf16, isOutput=False)
    qkb = nc.declare_dram_parameter("qkb", [128, 2 * NDT], f32, isOutput=False)
    vbb = nc.declare_dram_parameter("vbb", [128, D], bf16, isOutput=False)
    obb = nc.declare_dram_parameter("obb", [128, D], bf16, isOutput=False)
    out = nc.declare_dram_parameter("out", [P, D], bf16, isOutput=True)

    with tile.TileContext(nc) as tc:
        _emit(tc, nc, tile, mybir, f32, bf16, f8,
              tsh, tsl, llmh, llml, qkw, vwh, owT, qkb, vbb, obb, out)
    nc.compile()
    return nc


def _emit(tc, nc, tile, mybir, f32, bf16, f8,
          tsh, tsl, llmh, llml, qkw, vwh, owT, qkb, vbb, obb, out):
    from contextlib import ExitStack
    from concourse.masks import make_identity

    Exp = mybir.ActivationFunctionType.Exp
    DR = mybir.MatmulPerfMode.DoubleRow
    MUL = mybir.AluOpType.mult
    ADD = mybir.AluOpType.add
    EXP_SCALE = 0.125 / 64.0   # scores carry x8 * x8 from the fp8 stores

    with ExitStack() as ctx:
        persist = ctx.enter_context(tc.tile_pool(name="persist", bufs=1))
        ktpool = ctx.enter_context(tc.tile_pool(name="ktpool", bufs=3))
        qtpool = ctx.enter_context(tc.tile_pool(name="qtpool", bufs=3))
        expool = ctx.enter_context(tc.tile_pool(name="expool", bufs=20))
        rpool = ctx.enter_context(tc.tile_pool(name="rpool", bufs=2))
        opool = ctx.enter_context(tc.tile_pool(name="opool", bufs=5))

        ident = persist.tile([128, 128], bf16, name="ident", tag="ident")
        make_identity(nc, ident)

        # ---- persistent input tiles: ONE tile per tensor, DoubleRow view
        # [128, 8=(dp i), cols].  Element (p, 2dp+i, c) <- dram row
        # dp*256 + i*128 + p, col c.
        def big_tile(name, cols, dt=f8):
            t = persist.tile([128, 8 * cols], dt, name=name, tag=name)
            return t.rearrange("p (g c) -> p g c", g=8)

        ts_h3 = big_tile("ts_h", P)
        ts_l3 = big_tile("ts_l", P)
        llm_h3 = big_tile("llm_h", S)
        llm_l3 = big_tile("llm_l", S)
        qkw3 = big_tile("qkw", 2 * D)
        vw_h3 = big_tile("vw_h", D)
        qkb_sb = persist.tile([128, 2 * NDT], f32, name="qkb_sb", tag="qkb_sb")
        vbb_sb = persist.tile([128, D], bf16, name="vbb_sb", tag="vbb_sb")
        obb_sb = persist.tile([128, D], bf16, name="obb_sb", tag="obb_sb")
        # O weights: one tile, slice [:, d*1024 + jc*512 : ...]
        ow_flat = persist.tile([128, NDT * D], bf16, name="ow_sb", tag="ow_sb")

        def ow_sl(d, jc):
            return ow_flat[:, d * D + jc * 512:d * D + (jc + 1) * 512]

        # combined q/k weight column offsets within the 2048-col inner dim
        def qoff(b):
            return b * 128 if b < 2 else 512 + (b - 2) * 128

        def koff(b):
            return 256 + b * 128 if b < 2 else 1280 + (b - 2) * 128

        # ---- input DMAs: one big transfer each, sync queue only (the Act
        # queue must stay clear of DMA issues or they'd delay the exps;
        # HWDGE is a single shared device anyway).  Consumption order.
        def dma_big(dst3, dram, lo, hi):
            src = dram.ap()[:, lo:hi].rearrange("(g p) c -> p g c", g=8)
            nc.sync.dma_start(out=dst3[:, :, lo:hi], in_=src)

        dma_big(ts_h3, tsh, 0, P)
        nc.sync.dma_start(out=qkb_sb, in_=qkb.ap())
        dma_big(qkw3, qkw, 0, 512)          # q-j01 + k-j01 blocks
        dma_big(llm_h3, llmh, 0, 512)
        dma_big(ts_l3, tsl, 0, P)
        dma_big(llm_h3, llmh, 512, 1024)
        dma_big(vw_h3, vwh, 0, D)
        dma_big(llm_l3, llml, 0, 1024)
        dma_big(llm_h3, llmh, 1024, 2048)
        dma_big(llm_l3, llml, 1024, 2048)
        dma_big(qkw3, qkw, 512, 2048)       # q-rest + k-rest
        nc.sync.dma_start(out=vbb_sb, in_=vbb.ap())
        nc.sync.dma_start(out=obb_sb, in_=obb.ap())
        ow4 = ow_flat.rearrange("p (d j) -> p d j", j=D)
        for half in range(2):
            src = owT.ap()[half * 512:(half + 1) * 512, :] \
                .rearrange("(d p) j -> p d j", d=4)
            nc.sync.dma_start(out=ow4[:, half * 4:(half + 1) * 4, :], in_=src)

        # ---- on-chip intermediates ----
        # qt/kt land DIRECTLY in DoubleRow layout: the host permutes the
        # q/k weight columns so output partition q = jtsub*64+u*32+pr of
        # block (J,i) is head-dim j = (2J+jtsub)*128 + u*64 + i*32 + pr.
        # qt_dr[J] is [128, 2, P] (i-major halves), ktdr likewise over S.
        qt_dr = [None] * NDP      # [128, 2*P] f8, 8*q values
        kt_dr = [None] * (NDP + 1)  # [+1: full-precision redo of J0]
        vp_sb = [None] * NST      # [128, H*(DH+1)] bf16
        ctx_nat = persist.tile([128, NPT * D], bf16, name="ctx_nat",
                               tag="ctx_nat")
        cxT = []
        for d in range(NDT):
            cxT.append(persist.tile([128, P], bf16, name=f"cxT{d}",
                                    tag=f"cxT{d}"))
        partial = []
        for T in range(8):
            partial.append(persist.tile([128, 512], bf16, name=f"opart{T}",
                                        tag=f"opart{T}"))

        with tc.tile_pool(name="psS", bufs=2, space="PSUM") as psS, \
             tc.tile_pool(name="psC", bufs=2, space="PSUM") as psC, \
             tc.tile_pool(name="psP", bufs=2, space="PSUM") as psP:

            # ---------------- emission helpers ----------------
            def emit_qt(J, i, hi_only=False, dst_tile=None):
                # qt block (J,i) -> qt_dr[J] half i.  (ts_h+ts_l) x qw_h.
                # hi_only + dst_tile: startup variant into a scratch tile
                # (drops ts_l from the first-exp critical path; only score
                # stages (0,0)/(0,1) of heads 0/1 consume it).
                if dst_tile is None:
                    if qt_dr[J] is None:
                        qt_dr[J] = qtpool.tile([128, 2 * P], f8,
                                               name=f"qt_dr{J}", tag="qt")
                    dst_tile = qt_dr[J]
                b = 2 * J + i
                dst = dst_tile[:, i * P:(i + 1) * P]
                ps = psP.tile([128, P], f32, name=f"ps_q{b}", tag="psP")
                rhs_sets = (ts_h3,) if hi_only else (ts_h3, ts_l3)
                n = len(rhs_sets) * NDP
                g = 0
                for rhs3 in rhs_sets:
                    for dp in range(NDP):
                        nc.tensor.matmul(
                            ps,
                            lhsT=qkw3[:, 2 * dp:2 * dp + 2,
                                      qoff(b):qoff(b) + 128],
                            rhs=rhs3[:, 2 * dp:2 * dp + 2, :],
                            start=(g == 0), stop=(g == n - 1), perf_mode=DR)
                        g += 1
                # psum holds 16*q; store 8*q + 8*qb
                nc.vector.tensor_scalar(dst, ps, 0.5, qkb_sb[:, b:b + 1],
                                        MUL, ADD)

            def emit_kt_sc(slot, J, i, sc, lite=False):
                # kt block (J,i) s-chunk sc -> kt_dr[slot] half i.
                if kt_dr[slot] is None:
                    kt_dr[slot] = ktpool.tile([128, 2 * S], f8,
                                              name=f"kt_dr{slot}", tag="kt")
                b = 2 * J + i
                ps = psP.tile([128, 512], f32, name=f"ps_k{slot}_{b}_{sc}",
                              tag="psP")
                rhs_sets = (llm_h3,) if lite else (llm_h3, llm_l3)
                n = len(rhs_sets) * NDP
                g = 0
                for rhs3 in rhs_sets:
                    for dp in range(NDP):
                        nc.tensor.matmul(
                            ps,
                            lhsT=qkw3[:, 2 * dp:2 * dp + 2,
                                      koff(b):koff(b) + 128],
                            rhs=rhs3[:, 2 * dp:2 * dp + 2,
                                     sc * 512:(sc + 1) * 512],
                            start=(g == 0), stop=(g == n - 1), perf_mode=DR)
                        g += 1
                nc.vector.tensor_scalar(
                    kt_dr[slot][:, i * S + sc * 512:i * S + (sc + 1) * 512],
                    ps, 0.5, qkb_sb[:, NDT + b:NDT + b + 1], MUL, ADD)

            def emit_v(st, jc):
                # V'[s, h*65+x] bf16, heads jc*8..jc*8+8 only.  ctx for
                # pairs 0-3 reads just the jc=0 half, so jc=1 is deferred
                # to pairs 2-4.  psum = 16*v -> *1/16 + vb on evac.
                if vp_sb[st] is None:
                    vp_sb[st] = persist.tile([128, H * (DH + 1)], bf16,
                                             name=f"vp_sb{st}",
                                             tag=f"vp_sb{st}")
                vp3 = vp_sb[st].rearrange("p (h x) -> p h x", x=DH + 1)
                nc.gpsimd.memset(vp3[:, jc * 8:(jc + 1) * 8, DH:DH + 1], 1.0)
                ps = psP.tile([128, 512], f32, name=f"ps_v{st}_{jc}",
                              tag="psP")
                n = 2 * NDP
                i = 0
                for lhs3 in (llm_h3, llm_l3):
                    for dp in range(NDP):
                        nc.tensor.matmul(
                            ps,
                            lhsT=lhs3[:, 2 * dp:2 * dp + 2,
                                      st * 128:(st + 1) * 128],
                            rhs=vw_h3[:, 2 * dp:2 * dp + 2,
                                      jc * 512:(jc + 1) * 512],
                            start=(i == 0), stop=(i == n - 1),
                            perf_mode=DR)
                        i += 1
                nc.vector.scalar_tensor_tensor(
                    vp3[:, jc * 8:(jc + 1) * 8, 0:DH],
                    ps.rearrange("p (h x) -> p h x", x=DH),
                    1.0 / 16.0,
                    vbb_sb[:, jc * 512:(jc + 1) * 512]
                    .rearrange("p (h x) -> p h x", x=DH),
                    MUL, ADD)

            def emit_ctx(p, k, ets):
                # ctx[p-chunk, 0:65] += expT_h(st).T @ V'_h (bf16).
                for i in range(2):
                    st = 2 * k + i
                    for u in range(2):
                        h = 2 * p + u
                        for pc in range(NPT):
                            nc.tensor.matmul(
                                psc[u][:, pc * (DH + 1):
                                       (pc + 1) * (DH + 1)],
                                lhsT=ets[u][:, i * 512 + pc * 128:
                                            i * 512 + (pc + 1) * 128],
                                rhs=vp_sb[st][:, h * (DH + 1):
                                              (h + 1) * (DH + 1)],
                                start=(st == 0 and pc == 0),
                                stop=(st == NST - 1 and pc == NPT - 1))

            def emit_normalize(p, act_split=False):
                rcs = []
                for u in range(2):
                    h = 2 * p + u
                    psc3 = psc[u].rearrange("p (c x) -> p c x", x=DH + 1)
                    rc = rpool.tile([128, NPT], f32, name=f"rc{h}", tag="rc")
                    rc3 = rc.rearrange("p (c x) -> p c x", x=1)
                    nc.vector.reciprocal(rc3, psc3[:, :, DH:DH + 1])
                    rcs.append(rc)
                for pc in range(NPT):
                    for u in range(2):
                        h = 2 * p + u
                        dst = ctx_nat[:, pc * D + h * DH:pc * D + (h + 1) * DH]
                        srcp = psc[u][:, pc * (DH + 1):pc * (DH + 1) + DH]
                        if act_split and u == 1:
                            nc.scalar.mul(dst, srcp, rcs[u][:, pc:pc + 1])
                        else:
                            nc.vector.tensor_scalar_mul(
                                dst, srcp, rcs[u][:, pc:pc + 1])

            def emit_transposes(p, act_split=False):
                for pc in range(NPT):
                    pst = psP.tile([128, 128], bf16, name=f"pst{p}_{pc}",
                                   tag="psP")
                    nc.tensor.transpose(
                        pst,
                        ctx_nat[:, pc * D + p * 128:pc * D + (p + 1) * 128],
                        ident)
                    if act_split and pc % 2 == 1:
                        nc.scalar.copy(cxT[p][:, pc * 128:(pc + 1) * 128],
                                       pst)
                    else:
                        nc.vector.tensor_copy(
                            cxT[p][:, pc * 128:(pc + 1) * 128], pst)

            def emit_opartial(T):
                # out tile T=(pt,jc): bf16 sum d=0..5 plus output bias.
                pt, jc = T // 2, T % 2
                ps = psP.tile([128, 512], f32, name=f"ps_op{T}", tag="psP")
                for d in range(6):
                    nc.tensor.matmul(
                        ps, lhsT=cxT[d][:, pt * 128:(pt + 1) * 128],
                        rhs=ow_sl(d, jc),
                        start=(d == 0), stop=(d == 5))
                nc.vector.tensor_add(partial[T], ps,
                                     obb_sb[:, jc * 512:(jc + 1) * 512])

            # ---------------- prologue ----------------
            emit_qt(0, 0)
            emit_qt(0, 1)
            emit_kt_sc(0, 0, 0, 0, lite=True)
            emit_kt_sc(0, 0, 1, 0, lite=True)

            # ---- filler slot table: global stage -> emissions.  Balances
            # PE work against the 2.08us/stage Act budget; every entry is
            # placed >= ~2 stages before first consumption.
            sched = {}

            def at(g, fn, *args):
                sched.setdefault(g, []).append((fn, args))

            for j in range(12):               # V' heads 0-7: pairs 0-1
                at(j + 2, emit_v, j, 0)
            for j in (12, 13):
                at(14, emit_v, j, 0)
            for j in (14, 15):
                at(15, emit_v, j, 0)
            for j in range(16):               # V' heads 8-15: pairs 2-4
                at(16 + (11 * j) // 8, emit_v, j, 1)
            for s in (1, 2, 3):               # kt J0-lite rest (pair 0)
                at(2 * (s - 1), emit_kt_sc, 0, 0, 0, s, True)
                at(2 * (s - 1), emit_kt_sc, 0, 0, 1, s, True)
            for Jn in (1, 2, 3):              # kt J1..J3, JIT
                for s in range(4):
                    for i in range(2):
                        at(16 * Jn - 3 + 2 * s + i, emit_kt_sc,
                           Jn, Jn, i, s)
            at(5, emit_qt, 1, 0)              # qt J1..J3
            at(6, emit_qt, 1, 1)
            at(25, emit_qt, 2, 0)
            at(27, emit_qt, 2, 1)
            at(41, emit_qt, 3, 0)
            at(43, emit_qt, 3, 1)
            for T in range(6):                # O partials T0-5 in pair 7
                at(58 + T, emit_opartial, T)

            # ---------------- pipelined head pairs ----------------
            # ctx lags ONE PAIR (8 stages): pend queue of stage records.
            psc = None
            pend = []

            for p in range(NPAIR):
                for k in range(8):
                    g = 8 * p + k          # global stage index
                    if psc is None:
                        psc = [psC.tile([128, NPT * (DH + 1)], f32,
                                        name=f"psc{u}", tag="psC")
                               for u in range(2)]
                    pss = [psS.tile([128, 1024], f32,
                                    name=f"ps_s{2*p+u}_{k}", tag="psS")
                           for u in range(2)]
                    J, jsub = p // 2, p % 2
                    k3 = kt_dr[J].rearrange("q (i s) -> q i s", i=2)
                    q3 = qt_dr[J].rearrange("q (i x) -> q i x", i=2)
                    for u in range(2):
                        base = jsub * 64 + u * 32
                        for i in range(2):
                            st = 2 * k + i
                            nc.tensor.matmul(
                                pss[u][:, i * 512:(i + 1) * 512],
                                lhsT=k3[base:base + 32, :,
                                        st * 128:(st + 1) * 128],
                                rhs=q3[base:base + 32, :, :],
                                start=True, stop=True, perf_mode=DR,
                                tile_position=(base, 0))
                    ets = []
                    for u in range(2):
                        et = expool.tile([128, 1024], bf16,
                                         name=f"et{2*p+u}_{k}", tag="et")
                        nc.scalar.activation(et, pss[u], Exp,
                                             bias=0.0, scale=EXP_SCALE)
                        ets.append(et)

                    # ---- PE fillers from the slot table ----
                    for fn, args in sched.pop(g, ()):
                        fn(*args)
                    # transposes(p-2) once normalize(p-2) has run
                    if k == 1 and p >= 2:
                        emit_transposes(p - 2)

                    # ---- lagged ctx: one pair behind ----
                    pend.append((p, k, ets))
                    if len(pend) > 8:
                        cp, ck, cets = pend.pop(0)
                        emit_ctx(cp, ck, cets)
                        if ck == 7:
                            emit_normalize(cp)
                            psc = None

            # ---------------- tail ----------------
            # ctx(7,*) + T6/T7 partials interleaved, then normalize(7),
            # transposes(6,7), final d6/d7 + partial accumulate, out DMA.
            psc = [psC.tile([128, NPT * (DH + 1)], f32,
                            name=f"psc_t{u}", tag="psC") for u in range(2)]
            emit_opartial(6)
            emit_opartial(7)
            for idx in range(8):
                cp, ck, cets = pend.pop(0)
                emit_ctx(cp, ck, cets)
                if ck == 7:
                    emit_normalize(cp, act_split=True)
            emit_transposes(6, act_split=True)
            emit_transposes(7, act_split=True)
            for pc in range(NPT):
                ot = opool.tile([128, 1024], bf16, name=f"ot{pc}", tag="ot",
                                bufs=3)
                for jc in range(2):
                    T = pc * 2 + jc
                    tpool, ttag = ((psS, "psS") if jc == 0 else (psC, "psC"))
                    ps = tpool.tile([128, 512], f32, name=f"ps_o7_{T}",
                                    tag=ttag)
                    for d in (6, 7):
                        nc.tensor.matmul(
                            ps, lhsT=cxT[d][:, pc * 128:(pc + 1) * 128],
                            rhs=ow_sl(d, jc),
                            start=(d == 6), stop=False)
                    nc.tensor.matmul(ps, lhsT=ident, rhs=partial[T],
                                     start=False, stop=True)
                    if jc == 0:
                        nc.scalar.copy(ot[:, 0:512], ps)
                    else:
                        nc.vector.tensor_copy(ot[:, 512:1024], ps)
                    nc.sync.dma_start(
                        out=out.ap()[pc * 128:(pc + 1) * 128,
                                     jc * 512:(jc + 1) * 512],
                        in_=ot[:, jc * 512:(jc + 1) * 512])


def get_nc():
    global _cached_nc
    if _cached_nc is None:
        _cached_nc = _build_nc()
    return _cached_nc


def _split8(x):
    hi = x.astype(_F8)
    lo = (x - hi.astype(np.float32)).astype(_F8)
    return hi, lo


def _dr_perm():
    """Column permutation for q/k weights: new column b*128+q holds
    original head-dim j so the projection lands in DoubleRow layout.
    b = 2J+i, q = jtsub*64 + u*32 + pr -> j = (2J+jtsub)*128+u*64+i*32+pr.
    """
    c = np.arange(D)
    J, i, q = c // 256, (c % 256) // 128, c % 128
    jtsub, u, pr = q // 64, (q % 64) // 32, q % 32
    return (2 * J + jtsub) * 128 + u * 64 + i * 32 + pr


def make_in_maps(ts_features, llm_features, q_w, q_b, k_w, k_b, v_w, v_b,
                 o_w, o_b):
    ts = np.asarray(ts_features, np.float32)
    llm = np.asarray(llm_features, np.float32)
    qwT = np.ascontiguousarray(np.asarray(q_w, np.float32).T)
    kwT = np.ascontiguousarray(np.asarray(k_w, np.float32).T)
    vwT = np.ascontiguousarray(np.asarray(v_w, np.float32).T)
    owT = np.ascontiguousarray(np.asarray(o_w, np.float32).T)
    jmap = _dr_perm()
    qwP = (16.0 * qwT[:, jmap]).astype(_F8)
    kwP = (16.0 * kwT[:, jmap]).astype(_F8)
    # combined layout: [q-j01(256) | k-j01(256) | q-rest(768) | k-rest(768)]
    qkw = np.concatenate(
        [qwP[:, 0:256], kwP[:, 0:256], qwP[:, 256:1024], kwP[:, 256:1024]],
        axis=1)
    shared = {
        "qkw": np.ascontiguousarray(qkw),
        "vwh": np.ascontiguousarray((16.0 * vwT).astype(_F8)),
        "owT": owT.astype(_BF16),
        # biases for the x8-scaled fp8 qt/kt stores, in permuted order
        "qkb": np.ascontiguousarray(np.concatenate(
            [8.0 * np.asarray(q_b, np.float32)[jmap].reshape(NDT, 128).T,
             8.0 * np.asarray(k_b, np.float32)[jmap].reshape(NDT, 128).T],
            axis=1)),
        "vbb": np.ascontiguousarray(
            np.broadcast_to(np.asarray(v_b, np.float32), (128, D))).astype(_BF16),
        "obb": np.ascontiguousarray(
            np.broadcast_to(np.asarray(o_b, np.float32), (128, D))).astype(_BF16),
    }
    in_maps = []
    for b in range(NCORES):
        m = dict(shared)
        tsT = np.ascontiguousarray(ts[b].T)
        llmT = np.ascontiguousarray(llm[b].T)
        m["tsh"], m["tsl"] = _split8(tsT)
        m["llmh"], m["llml"] = _split8(llmT)
        in_maps.append(m)
    return in_maps


def kernel(**inputs):
    from concourse.bass_utils import run_bass_kernel_spmd

    nc = get_nc()
    in_maps = make_in_maps(**inputs)
    res = run_bass_kernel_spmd(nc, in_maps, list(range(NCORES)))
    return np.stack([res.results[i]["out"] for i in range(NCORES)],
                    axis=0).astype(np.float32)
